# revision 1
# baseline (speedup 1.0000x reference)
"""Multi-head causal attention (B=2, T=2048, C=1024, H=16, D=64) on 8 TRN2 cores.

Sharding: 2 heads per core (tensor-parallel over H). x is replicated (passed
pre-transposed as x^T so the contraction dim lands on SBUF partitions). Each
core computes y[:, :, 2c*64:(2c+2)*64]; host concatenates along channels.

Per-core dataflow (f32r matmuls everywhere except bf16 Q/K for scores;
f32r = full PE rate at N>=256 with ~11-bit-mantissa rounding):
  1. Projections, W stationary -> Q^T/K^T/V^T in [dd=2*64, t] layout (both
     heads stacked on partitions); scale 1/sqrt(C) folded into Wq on host.
     Q^T/K^T drain PSUM->SBUF as bf16, V^T as f32.
  2. V^T PE-transposed to V[s, d] per head with a ones column appended
     (V_aug[s, 65]) so the AV matmul also emits softmax sums for free.
  3. Scores S^T[s, t] = K^T(stationary) x Q^T(moving) per s-block, both
     heads paired in one 2-bank PSUM tile; columns below the causal
     diagonal are never computed (trimmed N).
  4. ONE exp call per s-block covers both heads PSUM->SBUF (f32r out, no
     max-subtraction needed: |scores| <= ~1); the diagonal 128x128 gets a
     multiplicative 0/1 triangle on DVE, off the ScalarE critical path.
  5. AV: V_aug stationary, E^T moving (N trimmed), accumulated over
     s-blocks in PSUM -> out^T[65, t] (row 64 = softmax sums).
  6. PE-transpose to [t, 65], DVE reciprocal of the sums column +
     per-partition scalar multiply, DMA out.

Schedule: one fused streaming pipeline per (b, t-tile); the NEXT tile's
projection work is emitted in closures interleaved between the current
tile's attention s-block periods, and AV lags scores by 2 s-blocks, so the
PE never idles (keeps the HAM clock gate at K=8/8) and ScalarE never
starves at tile boundaries.
"""

import numpy as np

import concourse.mybir as mybir
import concourse.tile as tile
from concourse import bacc
from concourse.masks import make_identity

B, T, C, H, D = 2, 2048, 1024, 16, 64
HPC = 2          # heads per core
NCORES = 8
TT = 512         # t-tile (moving free dim)
SB = 128         # s-block (scores stationary free dim)
NCH = C // 128   # contraction chunks for projections
F32 = mybir.dt.float32
F32R = mybir.dt.float32r
BF16 = mybir.dt.bfloat16


def build_nc(t_len=T, batches=B):
    nj = t_len // TT
    nc = bacc.Bacc("TRN2", target_bir_lowering=False, debug=False)
    xt = nc.dram_tensor("xt", [batches, C, t_len], F32R, kind="ExternalInput")
    wq = nc.dram_tensor("wq", [C, 2 * D], F32R, kind="ExternalInput")
    wk = nc.dram_tensor("wk", [C, 2 * D], F32R, kind="ExternalInput")
    wv = nc.dram_tensor("wv", [C, 2 * D], F32R, kind="ExternalInput")
    y = nc.dram_tensor("y", [batches, t_len, 2 * D], F32, kind="ExternalOutput")

    with tile.TileContext(nc) as tc:
        with (
            tc.tile_pool(name="consts", bufs=1) as consts,
            tc.tile_pool(name="wpool", bufs=1) as wpool,
            tc.tile_pool(name="qkv", bufs=batches) as qkv,
            tc.tile_pool(name="epool", bufs=4) as epool,
            tc.tile_pool(name="avs", bufs=2) as avs,
            tc.tile_pool(name="outp", bufs=8) as outp,
            tc.tile_pool(name="small", bufs=8) as small,
        ):
            identity = consts.tile([128, 128], F32)
            make_identity(nc, identity)
            # tri01[s, t_local] = 1 where t_local >= s else 0; multiplied
            # into the diagonal 128x128 sub-block of E after exp.
            tri01 = consts.tile([128, SB], F32R)
            nc.gpsimd.memset(tri01.bitcast(F32), 1.0)
            nc.gpsimd.affine_select(
                out=tri01.bitcast(F32), in_=tri01.bitcast(F32),
                compare_op=mybir.AluOpType.is_ge,
                fill=0.0, base=0,
                pattern=[[1, SB]], channel_multiplier=-1,
            )

            w_sb, w_src = {}, {}
            for name, w in (("q", wq), ("k", wk), ("v", wv)):
                wt = wpool.tile([128, NCH, 2 * D], F32R, tag=f"w{name}", name=f"w{name}_sb")
                w_sb[name] = wt
                w_src[name] = w

            # Persistent per-batch tensors
            QT, KT, VH = {}, {}, {}
            for b in range(batches):
                QT[b] = qkv.tile([128, t_len], BF16, tag="qt", name=f"qt{b}")
                KT[b] = qkv.tile([128, t_len], BF16, tag="kt", name=f"kt{b}")
                for h in range(HPC):
                    vh = qkv.tile([128, (t_len // SB) * (D + 1)], F32R, tag=f"vh{h}", name=f"vh{b}_{h}")
                    ones_view = vh.rearrange("p (i c) -> p i c", c=D + 1)[:, :, D:D + 1]
                    nc.gpsimd.memset(ones_view.bitcast(F32), 1.0)
                    VH[(b, h)] = vh

            # ---------------- fused streaming pipeline ----------------
            # Per (b, j): projections for t-tile j, then causal attention for
            # t-tile j (which only needs K/V up to tile j). One PSUM budget,
            # no phase boundary, so the PE stays continuously busy and the
            # HAM clock-gate stays warm. The attention inner loop software-
            # pipelines two head-streams with lag-1 AV so the PE never
            # stalls on exp.
            with (
                tc.tile_pool(name="xtp", bufs=3) as xtp,
                tc.tile_pool(name="vts", bufs=2) as vts,
                tc.tile_pool(name="mixps", bufs=2, space="PSUM") as mixps,
                tc.tile_pool(name="spsum", bufs=2, space="PSUM") as spsum,
                tc.tile_pool(name="avpsum", bufs=2, space="PSUM") as avpsum,
            ):
                def proj_closures(b, j):
                    """Projection work for (b, j) as a list of closures, to
                    be interleaved into the previous tile's attention
                    periods so neither PE nor ScalarE ever starves."""
                    state = {}

                    def do_load():
                        xr = xt[b].rearrange("(k p) t -> p k t", p=128)
                        xt_sb = xtp.tile([128, NCH, TT], F32R, tag="xts",
                                         name=f"xts{b}_{j}")
                        # two half-tile DMAs so the first proj matmuls only
                        # wait for the first half (cuts pipeline-fill)
                        half = NCH // 2
                        nc.sync.dma_start(
                            out=xt_sb[:, 0:half, :],
                            in_=xr[:, 0:half, j * TT:(j + 1) * TT])
                        nc.sync.dma_start(
                            out=xt_sb[:, half:, :],
                            in_=xr[:, half:, j * TT:(j + 1) * TT])
                        state["xt"] = xt_sb

                    def do_proj(name):
                        pp = mixps.tile([128, TT], F32, tag="mix",
                                        name=f"pp_{name}")
                        for kk in range(NCH):
                            nc.tensor.matmul(
                                pp,
                                lhsT=w_sb[name][:, kk, :],
                                rhs=state["xt"][:, kk, :],
                                start=(kk == 0), stop=(kk == NCH - 1),
                                skip_group_check=True,
                            )
                        if name == "q":
                            nc.vector.tensor_copy(
                                QT[b][:, j * TT:(j + 1) * TT], pp)
                        elif name == "k":
                            nc.vector.tensor_copy(
                                KT[b][:, j * TT:(j + 1) * TT], pp)
                        else:
                            vt_sb = vts.tile([128, TT], F32, tag="vt",
                                             name=f"vt{b}_{j}")
                            nc.vector.tensor_copy(vt_sb, pp)
                            state["vt"] = vt_sb

                    def do_vtrans(q4):
                        vp = mixps.tile([128, 128], F32, tag="mix",
                                        name=f"vp{q4}")
                        nc.tensor.transpose(
                            vp, state["vt"][:, q4 * 128:(q4 + 1) * 128],
                            identity)
                        sb = (j * TT) // SB + q4
                        for h in range(HPC):
                            nc.vector.tensor_copy(
                                VH[(b, h)][:, sb * (D + 1):sb * (D + 1) + D],
                                vp[:, h * D:(h + 1) * D])

                    ops = [lambda: do_proj("q"),
                           lambda: do_proj("k"),
                           lambda: do_proj("v")]
                    ops += [lambda q4=q4: do_vtrans(q4)
                            for q4 in range(TT // 128)]
                    return do_load, ops

                def emit_attention(b, j, pending):
                    """Causal attention for t-tile j. Per s-block: both
                    heads' score MMs into one paired PSUM tile [h0 | h1]
                    (disjoint row groups -> concurrent), ONE exp call for
                    both heads, multiplicative tri-mask on E after exp (off
                    the ACT critical path), AV lagging 2 s-blocks. Closures
                    in `pending` (next tile's projections) are drained
                    evenly across the periods."""
                    out_tiles = [outp.tile([128, 2 * D], F32, tag="out",
                                           name=f"out{b}_{j}_{q}")
                                 for q in range(TT // 128)]
                    n_sb = (j + 1) * TT // SB
                    av_ps = {h: avpsum.tile([D + 1, TT], F32, tag="avps",
                                            name=f"avps{h}")
                             for h in range(HPC)}
                    eg = {}
                    LAG = 2

                    def emit_scores(sb):
                        # off: columns below the causal diagonal are never
                        # computed (scores, exp, AV all trimmed to t >= s).
                        off = max(0, (sb - 4 * j) * SB)
                        S = spsum.tile([128, HPC * TT], F32,
                                       tag="spsum", name=f"s{sb}")
                        for h in range(HPC):
                            hp = slice(h * D, (h + 1) * D)
                            nc.tensor.matmul(
                                S[:, h * TT + off:(h + 1) * TT],
                                lhsT=KT[b][hp, sb * SB:(sb + 1) * SB],
                                rhs=QT[b][hp, j * TT + off:(j + 1) * TT],
                                start=True, stop=True,
                            )
                        e = epool.tile([128, HPC * TT], F32R, tag="e",
                                       name=f"e{sb}")
                        if off == 0:
                            nc.scalar.activation(
                                out=e, in_=S,
                                func=mybir.ActivationFunctionType.Exp)
                        else:
                            for h in range(HPC):
                                nc.scalar.activation(
                                    out=e[:, h * TT + off:(h + 1) * TT],
                                    in_=S[:, h * TT + off:(h + 1) * TT],
                                    func=mybir.ActivationFunctionType.Exp)
                        if sb >= 4 * j:  # diagonal triangle at cols [off, off+SB)
                            for h in range(HPC):
                                nc.vector.tensor_mul(
                                    e[:, h * TT + off:h * TT + off + SB],
                                    e[:, h * TT + off:h * TT + off + SB],
                                    tri01)
                        eg[sb] = (e, off)

                    def emit_av(sb):
                        e, off = eg.pop(sb)
                        for h in range(HPC):
                            nc.tensor.matmul(
                                av_ps[h][:, off:],
                                lhsT=VH[(b, h)][:, sb * (D + 1):(sb + 1) * (D + 1)],
                                rhs=e[:, h * TT + off:(h + 1) * TT],
                                start=(sb == 0), stop=(sb == n_sb - 1),
                                skip_group_check=True,
                            )

                    n_periods = n_sb + LAG
                    n_pend = len(pending)
                    popped = 0
                    for sb in range(n_periods):
                        if sb < n_sb:
                            emit_scores(sb)
                        want = (n_pend * (sb + 1)) // n_periods
                        while popped < want:
                            pending[popped]()
                            popped += 1
                        if sb >= LAG:
                            emit_av(sb - LAG)
                    assert popped == n_pend

                    for h in range(HPC):
                        av_sb = avs.tile([D + 1, TT], F32, name=f"avsb{h}")
                        nc.vector.tensor_copy(av_sb, av_ps[h])
                        for q4 in range(TT // 128):
                            ot = mixps.tile([128, D + 1], F32, tag="mix",
                                            name=f"ot{h}_{q4}")
                            nc.tensor.transpose(
                                ot, av_sb[:, q4 * 128:(q4 + 1) * 128],
                                identity[0:D + 1, 0:D + 1])
                            rec = small.tile([128, 1], F32)
                            nc.vector.reciprocal(rec, ot[:, D:D + 1])
                            nc.vector.tensor_scalar_mul(
                                out_tiles[q4][:, h * D:(h + 1) * D],
                                ot[:, 0:D], rec)
                    for q4 in range(TT // 128):
                        t0 = j * TT + q4 * 128
                        nc.sync.dma_start(
                            out=y[b, t0:t0 + 128, :], in_=out_tiles[q4])

                seq = [(b, j) for b in range(batches) for j in range(nj)]
                # First tile's x^T load goes to the FRONT of the sync DMA
                # queue (before the W loads) so the first projection matmul
                # is gated only by its own transfers.
                ld0, ops0 = proj_closures(*seq[0])
                ld0()
                for name in ("q", "k", "v"):
                    nc.sync.dma_start(
                        out=w_sb[name],
                        in_=w_src[name].rearrange("(k p) d -> p k d", p=128))
                for op in ops0:
                    op()
                for idx, (b, j) in enumerate(seq):
                    if idx + 1 < len(seq):
                        ldn, opsn = proj_closures(*seq[idx + 1])
                        nxt = [ldn] + opsn
                    else:
                        nxt = []
                    emit_attention(b, j, nxt)

    nc.compile()
    return nc


_CACHE = {}


def _get_runner():
    if "run" in _CACHE:
        return _CACHE["run"]

    import jax
    from jax.experimental.shard_map import shard_map
    from jax.sharding import Mesh, PartitionSpec
    from concourse import bass2jax
    from concourse.bass2jax import _bass_exec_p, install_neuronx_cc_hook

    nc = build_nc()
    install_neuronx_cc_hook()

    partition_name = (nc.partition_id_tensor.name
                      if nc.partition_id_tensor else None)
    in_names, out_names, out_avals, zero_outs = [], [], [], []
    for alloc in nc.m.functions[0].allocations:
        if not isinstance(alloc, mybir.MemoryLocationSet):
            continue
        name = alloc.memorylocations[0].name
        if alloc.kind == "ExternalInput":
            if name != partition_name:
                in_names.append(name)
        elif alloc.kind == "ExternalOutput":
            out_names.append(name)
            shape = tuple(alloc.tensor_shape)
            dtype = mybir.dt.np(alloc.dtype)
            out_avals.append(jax.core.ShapedArray(shape, dtype))
            zero_outs.append(np.zeros(shape, dtype))
    n_params = len(in_names)
    n_outs = len(out_avals)
    all_names = in_names + out_names
    if partition_name is not None:
        all_names = all_names + [partition_name]
    donate = tuple(range(n_params, n_params + n_outs))

    def _body(*args):
        operands = list(args)
        if partition_name is not None:
            operands.append(bass2jax.partition_id_tensor())
        outs = _bass_exec_p.bind(
            *operands,
            out_avals=tuple(out_avals),
            in_names=tuple(all_names),
            out_names=tuple(out_names),
            lowering_input_output_aliases=(),
            sim_require_finite=True,
            sim_require_nnan=True,
            nc=nc,
        )
        return tuple(outs)

    devices = jax.devices()[:NCORES]
    mesh = Mesh(np.asarray(devices), ("core",))
    in_specs = (PartitionSpec("core"),) * (n_params + n_outs)
    out_specs = (PartitionSpec("core"),) * n_outs
    sharded = jax.jit(
        shard_map(_body, mesh=mesh, in_specs=in_specs, out_specs=out_specs,
                  check_rep=False),
        donate_argnums=donate, keep_unused=True,
    )

    runner = {
        "nc": nc,
        "all_names": all_names,
        "sharded": sharded,
        "in_names": in_names,
        "out_names": out_names,
        "out_avals": out_avals,
        "zero_outs": zero_outs,
    }
    _CACHE["run"] = runner
    return runner


def _shard_inputs(x, Wq, Wk, Wv):
    """Per-core input dicts. Host-side layout prep only."""
    scale = float(C) ** -0.5
    xt = np.ascontiguousarray(
        np.transpose(x, (0, 2, 1)).astype(np.float32))  # [B, C, T]
    maps = []
    for c in range(NCORES):
        h0 = HPC * c
        wq2 = np.ascontiguousarray(
            (np.concatenate([Wq[h0 + i] for i in range(HPC)], axis=1)
             * scale).astype(np.float32))
        wk2 = np.ascontiguousarray(
            np.concatenate([Wk[h0 + i] for i in range(HPC)], axis=1).astype(np.float32))
        wv2 = np.ascontiguousarray(
            np.concatenate([Wv[h0 + i] for i in range(HPC)], axis=1).astype(np.float32))
        maps.append({"xt": xt, "wq": wq2, "wk": wk2, "wv": wv2})
    return maps


def run_sharded(in_maps):
    """Run the 8-core NEFF once; returns list of per-core output dicts."""
    r = _get_runner()
    concat_in = [
        np.concatenate([in_maps[c][name] for c in range(NCORES)], axis=0)
        for name in r["in_names"]
    ]
    concat_zeros = [
        np.zeros((NCORES * z.shape[0], *z.shape[1:]), z.dtype)
        for z in r["zero_outs"]
    ]
    out_arrs = r["sharded"](*concat_in, *concat_zeros)
    return [
        {
            name: np.asarray(out_arrs[i]).reshape(
                NCORES, *r["out_avals"][i].shape)[c]
            for i, name in enumerate(r["out_names"])
        }
        for c in range(NCORES)
    ]


def kernel(x, Wq, Wk, Wv):
    in_maps = _shard_inputs(
        np.asarray(x, dtype=np.float32), np.asarray(Wq, dtype=np.float32),
        np.asarray(Wk, dtype=np.float32), np.asarray(Wv, dtype=np.float32))
    results = run_sharded(in_maps)
    return np.concatenate([results[c]["y"] for c in range(NCORES)], axis=2)



# revision 6
# speedup vs baseline: 1.1397x; 1.1397x over previous
"""Multi-head causal attention (B=2, T=2048, C=1024, H=16, D=64) on 8 TRN2 cores.

Sharding: core c = (batch b = c//4, head-group hg = c%4): 4 heads of one batch
per core (halves x DMA vs replicating both batches). Host concatenates heads /
batches and normalizes (divide by softmax sums) + transposes on the way out.

Per-core dataflow, all matmuls bf16 x bf16 -> f32 PSUM (fast FWL weight loads;
LDWEIGHTS hides under the previous matmul, unlike f32r self-loading):
  1. q/k projections: W chunk stationary [128c, 128hd], x^T moving [128c, 512t]
     -> Q^T/K^T [hd, t] bf16, head pair g on partitions (h even: 0-63,
     h odd: 64-127). Scale 1/sqrt(C) folded into Wq on host.
  2. v projection TRANSPOSED: x^T chunk stationary [128c, 128t], Wv moving
     [128c, 256hd] -> V[t, hd] directly (no PE transposes at all).
  3. Scores S^T[s, 2, t] per (s-block, pair): K^T stationary, Q^T moving; the
     two heads of a pair run CONCURRENTLY via row-tiling (contraction d=64 ->
     tile_position (0,0)/(64,0) auto-derived from base partitions). Columns
     below the causal diagonal never computed (off trim).
  4. exp: s-block 0 (plus a tunable share) on ScalarE (exact); the rest on DVE
     via a 1-op Schraudolph: E_bf16bits = int16(S * 128/ln2 + (127*128-c)).
     Short softmax rows (t<128) live entirely in s-block 0, so keeping that
     block exact pins max rel err at the bf16 floor (~3e-3, validated offline).
     Diagonal 128x128 gets a multiplicative 0/1 triangle (DVE).
  5. AV: V[s,d] stationary per head, E^T moving; the two heads of a pair run
     CONCURRENTLY via col-tiling (output partitions 0-63 / 64-127 of one PSUM
     bank, tile_position (0,0)/(0,64) auto). Output is av^T[d-pair, t] --
     already in the y^T layout, no transposes. Softmax sums via 4 concurrent
     M=1 col-tiled matmuls (ones stationary) into partitions {0,32,64,96}.
  6. Host: y = (yt / sums) per head, transpose, concat.

Schedule: fused streaming pipeline per t-tile; next tile's projection closures
drain into the current tile's attention periods; AV lags scores by 2 s-blocks.
PSUM: scores 2x2 banks + av 2x1 + sums 1 + proj 1 = exactly 8 banks.
"""

import numpy as np

import concourse.mybir as mybir
import concourse.tile as tile
from concourse import bacc

B, T, C, H, D = 2, 2048, 1024, 16, 64
HPC = 4          # heads per core
NPAIR = 2        # head pairs per core
NCORES = 8
TT = 512         # t-tile
SB = 128         # s-block
NCH = C // 128   # contraction chunks
NTB = T // 128   # t-blocks (v-proj granularity)
F32 = mybir.dt.float32
BF16 = mybir.dt.bfloat16
I16 = mybir.dt.int16

# exp(x) ~= bitcast_bf16(int16(x * 128/ln2 + (127*128 - c))): Schraudolph in
# bf16 bits. Scores are in [-1, 1] so the int is ~16065..16435: no overflow,
# no sign issues. c calibrated offline; end-to-end error is insensitive to c
# and to round-vs-floor convert semantics (validated in numpy).
SCH_K = float(2.0 ** 7 / np.log(2.0))
SCH_B = float(127 * 128 - 5.0)

# Fraction of each tile's s-blocks computed on DVE (rest on ScalarE). s-block
# 0 always stays on ScalarE (exact exp) to protect short softmax rows. Skewed
# toward late tiles where ScalarE would otherwise be the per-tile bottleneck.
DVE_FRAC = {0: 0.0, 1: 0.35, 2: 0.55, 3: 0.65}


def dve_sbs(j):
    """Evenly-spread set of s-blocks (never 0) DVE-assigned for tile j."""
    n_sb = 4 * (j + 1)
    want = int(round(DVE_FRAC[j] * n_sb))
    if want <= 0:
        return set()
    cands = list(range(1, n_sb))
    step = len(cands) / want
    return {cands[min(len(cands) - 1, int(i * step + step / 2))]
            for i in range(want)}


def build_nc():
    nj = T // TT
    nc = bacc.Bacc("TRN2", target_bir_lowering=False, debug=False)
    xt = nc.dram_tensor("xt", [C, T], BF16, kind="ExternalInput")
    wq = nc.dram_tensor("wq", [C, HPC * D], BF16, kind="ExternalInput")
    wk = nc.dram_tensor("wk", [C, HPC * D], BF16, kind="ExternalInput")
    wv = nc.dram_tensor("wv", [C, HPC * D], BF16, kind="ExternalInput")
    yt = nc.dram_tensor("yt", [HPC * D, T], F32, kind="ExternalOutput")
    sm = nc.dram_tensor("sm", [nj, HPC, TT], F32, kind="ExternalOutput")

    with tile.TileContext(nc) as tc:
        with (
            tc.tile_pool(name="consts", bufs=1) as consts,
            tc.tile_pool(name="wpool", bufs=1) as wpool,
            tc.tile_pool(name="qkv", bufs=1) as qkv,
            tc.tile_pool(name="xtp", bufs=2) as xtp,
            tc.tile_pool(name="epool", bufs=6) as epool,
            tc.tile_pool(name="ysb", bufs=4) as ysbp,
            tc.tile_pool(name="smsb", bufs=2) as smsbp,
            tc.tile_pool(name="mixps", bufs=1, space="PSUM") as mixps,
            tc.tile_pool(name="spsum", bufs=2, space="PSUM") as spsum,
            tc.tile_pool(name="avps", bufs=2, space="PSUM") as avps,
            tc.tile_pool(name="smps", bufs=1, space="PSUM") as smps,
        ):
            ones1 = consts.tile([128, 1], BF16)
            nc.vector.memset(ones1, 1.0)
            # tri01[s, g, t_local] = 1 where t_local >= s else 0 (bf16 via a
            # one-time f32 build + cast; applied to the diagonal block of E).
            tri_f32 = consts.tile([128, 2, SB], F32)
            nc.gpsimd.memset(tri_f32, 1.0)
            for g in range(NPAIR):
                nc.gpsimd.affine_select(
                    out=tri_f32[:, g, :], in_=tri_f32[:, g, :],
                    compare_op=mybir.AluOpType.is_ge,
                    fill=0.0, base=0,
                    pattern=[[1, SB]], channel_multiplier=-1,
                )
            tri01 = consts.tile([128, 2, SB], BF16)
            nc.vector.tensor_copy(tri01, tri_f32)

            wq_sb = wpool.tile([128, NCH, HPC * D], BF16, tag="wq")
            wk_sb = wpool.tile([128, NCH, HPC * D], BF16, tag="wk")
            wv_sb = wpool.tile([128, NCH, HPC * D], BF16, tag="wv")
            w_sb = {"q": wq_sb, "k": wk_sb}

            QT = qkv.tile([128, NPAIR, T], BF16, tag="qt")
            KT = qkv.tile([128, NPAIR, T], BF16, tag="kt")
            V = qkv.tile([128, NTB, HPC * D], BF16, tag="v")

            def proj_closures(j):
                state = {}

                def do_load():
                    xr = xt.rearrange("(k p) t -> p k t", p=128)
                    xt_sb = xtp.tile([128, NCH, TT], BF16, tag="xts",
                                     name=f"xts{j}")
                    half = NCH // 2
                    nc.sync.dma_start(
                        out=xt_sb[:, 0:half, :],
                        in_=xr[:, 0:half, j * TT:(j + 1) * TT])
                    nc.sync.dma_start(
                        out=xt_sb[:, half:, :],
                        in_=xr[:, half:, j * TT:(j + 1) * TT])
                    state["xt"] = xt_sb

                def do_qk(name, g):
                    pp = mixps.tile([128, TT], F32, tag="mix",
                                    name=f"pp_{name}{g}_{j}")
                    for kk in range(NCH):
                        nc.tensor.matmul(
                            pp,
                            lhsT=w_sb[name][:, kk, g * 128:(g + 1) * 128],
                            rhs=state["xt"][:, kk, :],
                            start=(kk == 0), stop=(kk == NCH - 1),
                            skip_group_check=True,
                        )
                    dst = QT if name == "q" else KT
                    nc.vector.tensor_copy(dst[:, g, j * TT:(j + 1) * TT], pp)

                def do_v(tb2):
                    vp = mixps.tile([128, TT], F32, tag="mix",
                                    name=f"vp{tb2}_{j}")
                    for half in range(2):
                        tl = tb2 * 2 + half
                        for kk in range(NCH):
                            nc.tensor.matmul(
                                vp[:, half * 256:(half + 1) * 256],
                                lhsT=state["xt"][:, kk, tl * 128:(tl + 1) * 128],
                                rhs=wv_sb[:, kk, :],
                                start=(kk == 0), stop=(kk == NCH - 1),
                                skip_group_check=True,
                            )
                    gtb = j * 4 + tb2 * 2
                    nc.vector.tensor_copy(
                        V[:, gtb:gtb + 2, :],
                        vp.rearrange("p (a b) -> p a b", a=2))

                ops = [lambda: do_qk("q", 0), lambda: do_qk("q", 1),
                       lambda: do_qk("k", 0), lambda: do_qk("k", 1),
                       lambda: do_v(0), lambda: do_v(1)]
                return do_load, ops

            def emit_attention(j, pending):
                n_sb = 4 * (j + 1)
                dset = dve_sbs(j)
                av_ps = {g: avps.tile([128, TT], F32, tag="avps",
                                      name=f"av{j}_{g}")
                         for g in range(NPAIR)}
                sm_ps = smps.tile([128, TT], F32, tag="smps", name=f"smps{j}")
                eg = {}
                LAG = 2

                def emit_scores(sb):
                    off = max(0, (sb - 4 * j) * SB)
                    for g in range(NPAIR):
                        S = spsum.tile([128, 2, TT], F32, tag="spsum",
                                       name=f"s{j}_{sb}_{g}")
                        for hh in range(2):
                            hp = slice(hh * 64, (hh + 1) * 64)
                            nc.tensor.matmul(
                                S[:, hh, off:],
                                lhsT=KT[hp, g, sb * SB:(sb + 1) * SB],
                                rhs=QT[hp, g, j * TT + off:(j + 1) * TT],
                                start=True, stop=True,
                            )
                        e = epool.tile([128, 2, TT], BF16, tag="e",
                                       name=f"e{j}_{sb}_{g}")
                        if sb in dset:
                            nc.vector.tensor_scalar(
                                e.bitcast(I16)[:, :, off:],
                                S[:, :, off:],
                                SCH_K, SCH_B,
                                mybir.AluOpType.mult, mybir.AluOpType.add)
                        else:
                            nc.scalar.activation(
                                out=e[:, :, off:], in_=S[:, :, off:],
                                func=mybir.ActivationFunctionType.Exp)
                        if sb >= 4 * j:  # diagonal block: causal triangle
                            nc.vector.tensor_mul(
                                e[:, :, off:off + SB],
                                e[:, :, off:off + SB], tri01)
                        eg[(sb, g)] = e

                def emit_av(sb):
                    off = max(0, (sb - 4 * j) * SB)
                    es = {g: eg.pop((sb, g)) for g in range(NPAIR)}
                    for g in range(NPAIR):
                        for hh in range(2):
                            h = 2 * g + hh
                            nc.tensor.matmul(
                                av_ps[g][hh * 64:(hh + 1) * 64, off:],
                                lhsT=V[:, sb, h * 64:(h + 1) * 64],
                                rhs=es[g][:, hh, off:],
                                start=(sb == 0), stop=(sb == n_sb - 1),
                                skip_group_check=True,
                            )
                    for g in range(NPAIR):
                        for hh in range(2):
                            h = 2 * g + hh
                            nc.tensor.matmul(
                                sm_ps[32 * h:32 * h + 1, off:],
                                lhsT=ones1,
                                rhs=es[g][:, hh, off:],
                                start=(sb == 0), stop=(sb == n_sb - 1),
                                skip_group_check=True,
                                tile_position=(0, 32 * h),
                            )

                n_periods = n_sb + LAG
                n_pend = len(pending)
                popped = 0
                for sb in range(n_periods):
                    if sb < n_sb:
                        emit_scores(sb)
                    want = (n_pend * (sb + 1)) // n_periods
                    while popped < want:
                        pending[popped]()
                        popped += 1
                    if sb >= LAG:
                        emit_av(sb - LAG)
                assert popped == n_pend

                for g in range(NPAIR):
                    y_sb = ysbp.tile([128, TT], F32, tag="ysb",
                                     name=f"y{j}_{g}")
                    nc.vector.tensor_copy(y_sb, av_ps[g])
                    nc.sync.dma_start(
                        out=yt[g * 128:(g + 1) * 128, j * TT:(j + 1) * TT],
                        in_=y_sb)
                # sums live on strided partitions {0,32,64,96}; engine APs
                # must start on 32-aligned partitions, so compact onto
                # partition 0 along the free dim before the DMA.
                sm_sb = smsbp.tile([1, HPC, TT], F32, tag="smsb",
                                   name=f"sm{j}")
                for h in range(HPC):
                    nc.vector.tensor_copy(
                        sm_sb[:, h, :], sm_ps[32 * h:32 * h + 1, :])
                nc.sync.dma_start(out=sm[j:j + 1], in_=sm_sb)

            ld0, ops0 = proj_closures(0)
            ld0()
            for w_tile, w_dram in ((wq_sb, wq), (wk_sb, wk), (wv_sb, wv)):
                nc.sync.dma_start(
                    out=w_tile,
                    in_=w_dram.rearrange("(k p) d -> p k d", p=128))
            for op in ops0:
                op()
            for j in range(nj):
                if j + 1 < nj:
                    ldn, opsn = proj_closures(j + 1)
                    nxt = [ldn] + opsn
                else:
                    nxt = []
                emit_attention(j, nxt)

    nc.compile()
    return nc


_CACHE = {}


def _get_runner():
    if "run" in _CACHE:
        return _CACHE["run"]

    import jax
    from jax.experimental.shard_map import shard_map
    from jax.sharding import Mesh, PartitionSpec
    from concourse import bass2jax
    from concourse.bass2jax import _bass_exec_p, install_neuronx_cc_hook

    nc = build_nc()
    install_neuronx_cc_hook()

    partition_name = (nc.partition_id_tensor.name
                      if nc.partition_id_tensor else None)
    in_names, out_names, out_avals, zero_outs = [], [], [], []
    for alloc in nc.m.functions[0].allocations:
        if not isinstance(alloc, mybir.MemoryLocationSet):
            continue
        name = alloc.memorylocations[0].name
        if alloc.kind == "ExternalInput":
            if name != partition_name:
                in_names.append(name)
        elif alloc.kind == "ExternalOutput":
            out_names.append(name)
            shape = tuple(alloc.tensor_shape)
            dtype = mybir.dt.np(alloc.dtype)
            out_avals.append(jax.core.ShapedArray(shape, dtype))
            zero_outs.append(np.zeros(shape, dtype))
    n_params = len(in_names)
    n_outs = len(out_avals)
    all_names = in_names + out_names
    if partition_name is not None:
        all_names = all_names + [partition_name]
    donate = tuple(range(n_params, n_params + n_outs))

    def _body(*args):
        operands = list(args)
        if partition_name is not None:
            operands.append(bass2jax.partition_id_tensor())
        outs = _bass_exec_p.bind(
            *operands,
            out_avals=tuple(out_avals),
            in_names=tuple(all_names),
            out_names=tuple(out_names),
            lowering_input_output_aliases=(),
            sim_require_finite=True,
            sim_require_nnan=True,
            nc=nc,
        )
        return tuple(outs)

    devices = jax.devices()[:NCORES]
    mesh = Mesh(np.asarray(devices), ("core",))
    in_specs = (PartitionSpec("core"),) * (n_params + n_outs)
    out_specs = (PartitionSpec("core"),) * n_outs
    sharded = jax.jit(
        shard_map(_body, mesh=mesh, in_specs=in_specs, out_specs=out_specs,
                  check_rep=False),
        donate_argnums=donate, keep_unused=True,
    )

    runner = {
        "nc": nc,
        "all_names": all_names,
        "sharded": sharded,
        "in_names": in_names,
        "out_names": out_names,
        "out_avals": out_avals,
        "zero_outs": zero_outs,
    }
    _CACHE["run"] = runner
    return runner


def _shard_inputs(x, Wq, Wk, Wv):
    """Per-core input dicts. Host-side layout prep only."""
    bf = mybir.dt.np(BF16)
    scale = float(C) ** -0.5
    maps = []
    for c in range(NCORES):
        b, hg = divmod(c, 4)
        hs = list(range(HPC * hg, HPC * hg + HPC))
        xtb = np.ascontiguousarray(
            np.transpose(x[b]).astype(bf))  # [C, T] bf16
        wq2 = np.ascontiguousarray(
            (np.concatenate([Wq[h] for h in hs], axis=1) * scale).astype(bf))
        wk2 = np.ascontiguousarray(
            np.concatenate([Wk[h] for h in hs], axis=1).astype(bf))
        wv2 = np.ascontiguousarray(
            np.concatenate([Wv[h] for h in hs], axis=1).astype(bf))
        maps.append({"xt": xtb, "wq": wq2, "wk": wk2, "wv": wv2})
    return maps


def run_sharded(in_maps):
    """Run the 8-core NEFF once; returns list of per-core output dicts."""
    r = _get_runner()
    concat_in = [
        np.concatenate([in_maps[c][name] for c in range(NCORES)], axis=0)
        for name in r["in_names"]
    ]
    concat_zeros = [
        np.zeros((NCORES * z.shape[0], *z.shape[1:]), z.dtype)
        for z in r["zero_outs"]
    ]
    out_arrs = r["sharded"](*concat_in, *concat_zeros)
    return [
        {
            name: np.asarray(out_arrs[i]).reshape(
                NCORES, *r["out_avals"][i].shape)[c]
            for i, name in enumerate(r["out_names"])
        }
        for c in range(NCORES)
    ]


def kernel(x, Wq, Wk, Wv):
    x = np.asarray(x, dtype=np.float32)
    Wq = np.asarray(Wq, dtype=np.float32)
    Wk = np.asarray(Wk, dtype=np.float32)
    Wv = np.asarray(Wv, dtype=np.float32)
    in_maps = _shard_inputs(x, Wq, Wk, Wv)
    results = run_sharded(in_maps)
    outs = []
    for b in range(B):
        parts = []
        for hg in range(4):
            r = results[b * 4 + hg]
            ytc = np.asarray(r["yt"], dtype=np.float32)   # [256, T]
            smc = np.asarray(r["sm"], dtype=np.float32)   # [nj, 4, TT]
            smc = smc.transpose(1, 0, 2).reshape(HPC, T)  # [4, T]
            yn = ytc.reshape(HPC, D, T) / smc[:, None, :]
            parts.append(yn.reshape(HPC * D, T).T)        # [T, 256]
        outs.append(np.concatenate(parts, axis=1))        # [T, 1024]
    return np.ascontiguousarray(np.stack(outs)).astype(np.float32)


# revision 12
# speedup vs baseline: 1.1625x; 1.0200x over previous
"""Multi-head causal attention (B=2, T=2048, C=1024, H=16, D=64) on 8 TRN2 cores.

Sharding: core c = (batch b = c//4, head-group hg = c%4): 4 heads of one batch
per core (halves x DMA vs replicating both batches). Host concatenates heads /
batches and normalizes (divide by softmax sums) + transposes on the way out.

Per-core dataflow, all matmuls bf16 x bf16 -> f32 PSUM (fast FWL weight loads;
LDWEIGHTS hides under the previous matmul, unlike f32r self-loading):
  1. q/k projections: W chunk stationary [128c, 128hd], x^T moving [128c, 512t]
     -> Q^T/K^T [hd, t] bf16, head pair g on partitions (h even: 0-63,
     h odd: 64-127). Scale 1/sqrt(C) folded into Wq on host.
  2. v projection TRANSPOSED: x^T chunk stationary [128c, 128t], Wv moving
     [128c, 256hd] -> V[t, hd] directly (no PE transposes at all).
  3. Scores S^T[s, 2, t] per (s-block, pair): K^T stationary, Q^T moving; the
     two heads of a pair run CONCURRENTLY via row-tiling (contraction d=64 ->
     tile_position (0,0)/(64,0) auto-derived from base partitions). Columns
     below the causal diagonal never computed (off trim).
  4. exp: s-block 0 (plus a tunable share) on ScalarE (exact); the rest on DVE
     via a 1-op Schraudolph: E_bf16bits = int16(S * 128/ln2 + (127*128-c)).
     Short softmax rows (t<128) live entirely in s-block 0, so keeping that
     block exact pins max rel err at the bf16 floor (~3e-3, validated offline).
     Diagonal 128x128 gets a multiplicative 0/1 triangle (DVE).
  5. AV: V[s,d] stationary per head, E^T moving; the two heads of a pair run
     CONCURRENTLY via col-tiling (output partitions 0-63 / 64-127 of one PSUM
     bank, tile_position (0,0)/(0,64) auto). Output is av^T[d-pair, t] --
     already in the y^T layout, no transposes. Softmax sums via 4 concurrent
     M=1 col-tiled matmuls (ones stationary) into partitions {0,32,64,96}.
  6. Host: y = (yt / sums) per head, transpose, concat.

Schedule: fused streaming pipeline per t-tile; next tile's projection closures
drain into the current tile's attention periods; AV lags scores by 2 s-blocks.
PSUM: scores 2x2 banks + av 2x1 + sums 1 + proj 1 = exactly 8 banks.
"""

import numpy as np

import concourse.mybir as mybir
import concourse.tile as tile
from concourse import bacc

B, T, C, H, D = 2, 2048, 1024, 16, 64
HPC = 4          # heads per core
NPAIR = 2        # head pairs per core
NCORES = 8
TT = 512         # t-tile
SB = 128         # s-block
NCH = C // 128   # contraction chunks
NTB = T // 128   # t-blocks (v-proj granularity)
F32 = mybir.dt.float32
BF16 = mybir.dt.bfloat16
I16 = mybir.dt.int16

# exp(x) ~= bitcast_bf16(int16(x * 128/ln2 + (127*128 - c))): Schraudolph in
# bf16 bits. Scores are in [-1, 1] so the int is ~16065..16435: no overflow,
# no sign issues. c calibrated offline; end-to-end error is insensitive to c
# and to round-vs-floor convert semantics (validated in numpy).
SCH_K = float(2.0 ** 7 / np.log(2.0))
SCH_B = float(127 * 128 - 5.0)

# exp engine assignment: s-block 0 goes to ScalarE for BOTH pairs (exact exp
# protects short softmax rows); for sb>0 the two pairs split across ScalarE
# and DVE (alternating by sb so each head sees a ~50/50 exact/approx mix and
# each S-buffer's release chain alternates engines -> no single-engine
# backlog stalls the scores matmuls on the PSUM WAR).


def build_nc():
    nj = T // TT
    nc = bacc.Bacc("TRN2", target_bir_lowering=False, debug=False)
    xt = nc.dram_tensor("xt", [C, T], BF16, kind="ExternalInput")
    wq = nc.dram_tensor("wq", [C, HPC * D], BF16, kind="ExternalInput")
    wk = nc.dram_tensor("wk", [C, HPC * D], BF16, kind="ExternalInput")
    wv = nc.dram_tensor("wv", [C, HPC * D], BF16, kind="ExternalInput")
    yt = nc.dram_tensor("yt", [HPC * D, T], F32, kind="ExternalOutput")
    sm = nc.dram_tensor("sm", [nj, HPC, TT], F32, kind="ExternalOutput")

    with tile.TileContext(nc) as tc:
        with (
            tc.tile_pool(name="consts", bufs=1) as consts,
            tc.tile_pool(name="wpool", bufs=1) as wpool,
            tc.tile_pool(name="qkv", bufs=1) as qkv,
            tc.tile_pool(name="xtp", bufs=2) as xtp,
            tc.tile_pool(name="epool", bufs=6) as epool,
            tc.tile_pool(name="ysb", bufs=4) as ysbp,
            tc.tile_pool(name="smsb", bufs=2) as smsbp,
            tc.tile_pool(name="mixps", bufs=1, space="PSUM") as mixps,
            tc.tile_pool(name="spsum", bufs=2, space="PSUM") as spsum,
            tc.tile_pool(name="avps", bufs=2, space="PSUM") as avps,
            tc.tile_pool(name="smps", bufs=1, space="PSUM") as smps,
        ):
            ones1 = consts.tile([128, 1], BF16)
            nc.vector.memset(ones1, 1.0)
            # tri01[s, g, t_local] = 1 where t_local >= s else 0 (bf16 via a
            # one-time f32 build + cast; applied to the diagonal block of E).
            tri_f32 = consts.tile([128, 2, SB], F32)
            nc.gpsimd.memset(tri_f32, 1.0)
            for g in range(NPAIR):
                nc.gpsimd.affine_select(
                    out=tri_f32[:, g, :], in_=tri_f32[:, g, :],
                    compare_op=mybir.AluOpType.is_ge,
                    fill=0.0, base=0,
                    pattern=[[1, SB]], channel_multiplier=-1,
                )
            tri01 = consts.tile([128, 2, SB], BF16)
            nc.vector.tensor_copy(tri01, tri_f32)

            wq_sb = wpool.tile([128, NCH, HPC * D], BF16, tag="wq")
            wk_sb = wpool.tile([128, NCH, HPC * D], BF16, tag="wk")
            wv_sb = wpool.tile([128, NCH, HPC * D], BF16, tag="wv")
            w_sb = {"q": wq_sb, "k": wk_sb}

            QT = qkv.tile([128, NPAIR, T], BF16, tag="qt")
            KT = qkv.tile([128, NPAIR, T], BF16, tag="kt")
            V = qkv.tile([128, NTB, HPC * D], BF16, tag="v")

            def proj_closures(j):
                state = {}

                def do_load():
                    xr = xt.rearrange("(k p) t -> p k t", p=128)
                    xt_sb = xtp.tile([128, NCH, TT], BF16, tag="xts",
                                     name=f"xts{j}")
                    half = NCH // 2
                    nc.sync.dma_start(
                        out=xt_sb[:, 0:half, :],
                        in_=xr[:, 0:half, j * TT:(j + 1) * TT])
                    nc.sync.dma_start(
                        out=xt_sb[:, half:, :],
                        in_=xr[:, half:, j * TT:(j + 1) * TT])
                    state["xt"] = xt_sb

                def do_qk(name, g):
                    pp = mixps.tile([128, TT], F32, tag="mix",
                                    name=f"pp_{name}{g}_{j}")
                    for kk in range(NCH):
                        nc.tensor.matmul(
                            pp,
                            lhsT=w_sb[name][:, kk, g * 128:(g + 1) * 128],
                            rhs=state["xt"][:, kk, :],
                            start=(kk == 0), stop=(kk == NCH - 1),
                            skip_group_check=True,
                        )
                    dst = QT if name == "q" else KT
                    nc.vector.tensor_copy(dst[:, g, j * TT:(j + 1) * TT], pp)

                def do_v(tb2):
                    vp = mixps.tile([128, TT], F32, tag="mix",
                                    name=f"vp{tb2}_{j}")
                    for half in range(2):
                        tl = tb2 * 2 + half
                        for kk in range(NCH):
                            nc.tensor.matmul(
                                vp[:, half * 256:(half + 1) * 256],
                                lhsT=state["xt"][:, kk, tl * 128:(tl + 1) * 128],
                                rhs=wv_sb[:, kk, :],
                                start=(kk == 0), stop=(kk == NCH - 1),
                                skip_group_check=True,
                            )
                    gtb = j * 4 + tb2 * 2
                    nc.vector.tensor_copy(
                        V[:, gtb:gtb + 2, :],
                        vp.rearrange("p (a b) -> p a b", a=2))

                ops = [lambda: do_qk("q", 0), lambda: do_qk("q", 1),
                       lambda: do_qk("k", 0), lambda: do_qk("k", 1),
                       lambda: do_v(0), lambda: do_v(1)]
                return do_load, ops

            def emit_attention(j, pending):
                n_sb = 4 * (j + 1)
                av_ps = {g: avps.tile([128, TT], F32, tag="avps",
                                      name=f"av{j}_{g}")
                         for g in range(NPAIR)}
                sm_ps = smps.tile([128, TT], F32, tag="smps", name=f"smps{j}")
                eg = {}
                LAG = 2

                def emit_scores(sb):
                    off = max(0, (sb - 4 * j) * SB)
                    for g in range(NPAIR):
                        S = spsum.tile([128, 2, TT], F32, tag="spsum",
                                       name=f"s{j}_{sb}_{g}")
                        for hh in range(2):
                            hp = slice(hh * 64, (hh + 1) * 64)
                            nc.tensor.matmul(
                                S[:, hh, off:],
                                lhsT=KT[hp, g, sb * SB:(sb + 1) * SB],
                                rhs=QT[hp, g, j * TT + off:(j + 1) * TT],
                                start=True, stop=True,
                            )
                        e = epool.tile([128, 2, TT], BF16, tag="e",
                                       name=f"e{j}_{sb}_{g}")
                        if sb != 0 and g != (sb & 1):
                            nc.vector.tensor_scalar(
                                e.bitcast(I16)[:, :, off:],
                                S[:, :, off:],
                                SCH_K, SCH_B,
                                mybir.AluOpType.mult, mybir.AluOpType.add)
                        else:
                            nc.scalar.activation(
                                out=e[:, :, off:], in_=S[:, :, off:],
                                func=mybir.ActivationFunctionType.Exp)
                        if sb >= 4 * j:  # diagonal block: causal triangle
                            nc.vector.tensor_mul(
                                e[:, :, off:off + SB],
                                e[:, :, off:off + SB], tri01)
                        eg[(sb, g)] = e

                def emit_av(sb):
                    off = max(0, (sb - 4 * j) * SB)
                    es = {g: eg.pop((sb, g)) for g in range(NPAIR)}
                    for g in range(NPAIR):
                        for hh in range(2):
                            h = 2 * g + hh
                            nc.tensor.matmul(
                                av_ps[g][hh * 64:(hh + 1) * 64, off:],
                                lhsT=V[:, sb, h * 64:(h + 1) * 64],
                                rhs=es[g][:, hh, off:],
                                start=(sb == 0), stop=(sb == n_sb - 1),
                                skip_group_check=True,
                            )
                    for g in range(NPAIR):
                        for hh in range(2):
                            h = 2 * g + hh
                            nc.tensor.matmul(
                                sm_ps[32 * h:32 * h + 1, off:],
                                lhsT=ones1,
                                rhs=es[g][:, hh, off:],
                                start=(sb == 0), stop=(sb == n_sb - 1),
                                skip_group_check=True,
                                tile_position=(0, 32 * h),
                            )

                n_periods = n_sb + LAG
                n_pend = len(pending)
                popped = 0
                for sb in range(n_periods):
                    if sb < n_sb:
                        emit_scores(sb)
                    want = (n_pend * (sb + 1)) // n_periods
                    while popped < want:
                        pending[popped]()
                        popped += 1
                    if sb >= LAG:
                        emit_av(sb - LAG)
                assert popped == n_pend

                # output drains on ScalarE (idle vs DVE; closer to PSUM)
                for g in range(NPAIR):
                    y_sb = ysbp.tile([128, TT], F32, tag="ysb",
                                     name=f"y{j}_{g}")
                    nc.scalar.copy(y_sb, av_ps[g])
                    nc.sync.dma_start(
                        out=yt[g * 128:(g + 1) * 128, j * TT:(j + 1) * TT],
                        in_=y_sb)
                # sums live on strided partitions {0,32,64,96}; engine APs
                # must start on 32-aligned partitions, so compact onto
                # partition 0 along the free dim before the DMA.
                sm_sb = smsbp.tile([1, HPC, TT], F32, tag="smsb",
                                   name=f"sm{j}")
                for h in range(HPC):
                    nc.scalar.copy(
                        sm_sb[:, h, :], sm_ps[32 * h:32 * h + 1, :])
                nc.sync.dma_start(out=sm[j:j + 1], in_=sm_sb)

            ld0, ops0 = proj_closures(0)
            ld0()
            nc.sync.dma_start(
                out=wq_sb, in_=wq.rearrange("(k p) d -> p k d", p=128))
            sm_ps_warm = smps.tile([128, TT], F32, tag="smps", name="smwarm")
            # PE warm-up: ~5us of dependency-free tiny matmuls during the
            # DMA fill so the HAM clock-gate is at 8/8 when the first
            # projection matmul issues (cold K=4/8 costs 2x for ~10us).
            # Writes scratch into the sums bank; attention later overwrites
            # it via start=True.
            for _ in range(110):
                nc.tensor.matmul(sm_ps_warm[0:1, 0:1], lhsT=ones1,
                                 rhs=ones1, start=True, stop=True,
                                 skip_group_check=True)
            # q-proj right after the wq DMA so its weight loads wait only on
            # wq (not on wk/wv); then interleave the remaining W DMAs.
            ops0[0]()
            ops0[1]()
            nc.sync.dma_start(
                out=wk_sb, in_=wk.rearrange("(k p) d -> p k d", p=128))
            ops0[2]()
            ops0[3]()
            nc.sync.dma_start(
                out=wv_sb, in_=wv.rearrange("(k p) d -> p k d", p=128))
            ops0[4]()
            ops0[5]()
            for j in range(nj):
                if j + 1 < nj:
                    ldn, opsn = proj_closures(j + 1)
                    nxt = [ldn] + opsn
                else:
                    nxt = []
                emit_attention(j, nxt)

    nc.compile()
    return nc


_CACHE = {}


def _get_runner():
    if "run" in _CACHE:
        return _CACHE["run"]

    import jax
    from jax.experimental.shard_map import shard_map
    from jax.sharding import Mesh, PartitionSpec
    from concourse import bass2jax
    from concourse.bass2jax import _bass_exec_p, install_neuronx_cc_hook

    nc = build_nc()
    install_neuronx_cc_hook()

    partition_name = (nc.partition_id_tensor.name
                      if nc.partition_id_tensor else None)
    in_names, out_names, out_avals, zero_outs = [], [], [], []
    for alloc in nc.m.functions[0].allocations:
        if not isinstance(alloc, mybir.MemoryLocationSet):
            continue
        name = alloc.memorylocations[0].name
        if alloc.kind == "ExternalInput":
            if name != partition_name:
                in_names.append(name)
        elif alloc.kind == "ExternalOutput":
            out_names.append(name)
            shape = tuple(alloc.tensor_shape)
            dtype = mybir.dt.np(alloc.dtype)
            out_avals.append(jax.core.ShapedArray(shape, dtype))
            zero_outs.append(np.zeros(shape, dtype))
    n_params = len(in_names)
    n_outs = len(out_avals)
    all_names = in_names + out_names
    if partition_name is not None:
        all_names = all_names + [partition_name]
    donate = tuple(range(n_params, n_params + n_outs))

    def _body(*args):
        operands = list(args)
        if partition_name is not None:
            operands.append(bass2jax.partition_id_tensor())
        outs = _bass_exec_p.bind(
            *operands,
            out_avals=tuple(out_avals),
            in_names=tuple(all_names),
            out_names=tuple(out_names),
            lowering_input_output_aliases=(),
            sim_require_finite=True,
            sim_require_nnan=True,
            nc=nc,
        )
        return tuple(outs)

    devices = jax.devices()[:NCORES]
    mesh = Mesh(np.asarray(devices), ("core",))
    in_specs = (PartitionSpec("core"),) * (n_params + n_outs)
    out_specs = (PartitionSpec("core"),) * n_outs
    sharded = jax.jit(
        shard_map(_body, mesh=mesh, in_specs=in_specs, out_specs=out_specs,
                  check_rep=False),
        donate_argnums=donate, keep_unused=True,
    )

    runner = {
        "nc": nc,
        "all_names": all_names,
        "sharded": sharded,
        "in_names": in_names,
        "out_names": out_names,
        "out_avals": out_avals,
        "zero_outs": zero_outs,
    }
    _CACHE["run"] = runner
    return runner


def _shard_inputs(x, Wq, Wk, Wv):
    """Per-core input dicts. Host-side layout prep only."""
    bf = mybir.dt.np(BF16)
    scale = float(C) ** -0.5
    maps = []
    for c in range(NCORES):
        b, hg = divmod(c, 4)
        hs = list(range(HPC * hg, HPC * hg + HPC))
        xtb = np.ascontiguousarray(
            np.transpose(x[b]).astype(bf))  # [C, T] bf16
        wq2 = np.ascontiguousarray(
            (np.concatenate([Wq[h] for h in hs], axis=1) * scale).astype(bf))
        wk2 = np.ascontiguousarray(
            np.concatenate([Wk[h] for h in hs], axis=1).astype(bf))
        wv2 = np.ascontiguousarray(
            np.concatenate([Wv[h] for h in hs], axis=1).astype(bf))
        maps.append({"xt": xtb, "wq": wq2, "wk": wk2, "wv": wv2})
    return maps


def run_sharded(in_maps):
    """Run the 8-core NEFF once; returns list of per-core output dicts."""
    r = _get_runner()
    concat_in = [
        np.concatenate([in_maps[c][name] for c in range(NCORES)], axis=0)
        for name in r["in_names"]
    ]
    concat_zeros = [
        np.zeros((NCORES * z.shape[0], *z.shape[1:]), z.dtype)
        for z in r["zero_outs"]
    ]
    out_arrs = r["sharded"](*concat_in, *concat_zeros)
    return [
        {
            name: np.asarray(out_arrs[i]).reshape(
                NCORES, *r["out_avals"][i].shape)[c]
            for i, name in enumerate(r["out_names"])
        }
        for c in range(NCORES)
    ]


def kernel(x, Wq, Wk, Wv):
    x = np.asarray(x, dtype=np.float32)
    Wq = np.asarray(Wq, dtype=np.float32)
    Wk = np.asarray(Wk, dtype=np.float32)
    Wv = np.asarray(Wv, dtype=np.float32)
    in_maps = _shard_inputs(x, Wq, Wk, Wv)
    results = run_sharded(in_maps)
    outs = []
    for b in range(B):
        parts = []
        for hg in range(4):
            r = results[b * 4 + hg]
            ytc = np.asarray(r["yt"], dtype=np.float32)   # [256, T]
            smc = np.asarray(r["sm"], dtype=np.float32)   # [nj, 4, TT]
            smc = smc.transpose(1, 0, 2).reshape(HPC, T)  # [4, T]
            yn = ytc.reshape(HPC, D, T) / smc[:, None, :]
            parts.append(yn.reshape(HPC * D, T).T)        # [T, 256]
        outs.append(np.concatenate(parts, axis=1))        # [T, 1024]
    return np.ascontiguousarray(np.stack(outs)).astype(np.float32)


# revision 21
# speedup vs baseline: 1.2360x; 1.0632x over previous
"""Multi-head causal attention (B=2, T=2048, C=1024, H=16, D=64) on 8 TRN2 cores.

Sharding: core c = (batch b = c//4, head-group hg = c%4): 4 heads of one batch
per core (halves x DMA vs replicating both batches). Host concatenates heads /
batches and normalizes (divide by softmax sums) + transposes on the way out.

Per-core dataflow, all matmuls bf16 x bf16 -> f32 PSUM (fast FWL weight loads;
LDWEIGHTS hides under the previous matmul, unlike f32r self-loading):
  1. q/k projections: W chunk stationary [128c, 128hd], x^T moving [128c, 512t]
     -> Q^T/K^T [hd, t] bf16, head pair g on partitions (h even: 0-63,
     h odd: 64-127). Scale 1/sqrt(C) folded into Wq on host.
  2. v projection TRANSPOSED: x^T chunk stationary [128c, 128t], Wv moving
     [128c, 256hd] -> V[t, hd] directly (no PE transposes at all).
  3. Scores S^T[s, 2, t] per (s-block, pair): K^T stationary, Q^T moving; the
     two heads of a pair run CONCURRENTLY via row-tiling (contraction d=64 ->
     tile_position (0,0)/(64,0) auto-derived from base partitions). Columns
     below the causal diagonal never computed (off trim).
  4. exp: s-block 0 (plus a tunable share) on ScalarE (exact); the rest on DVE
     via a 1-op Schraudolph: E_bf16bits = int16(S * 128/ln2 + (127*128-c)).
     Short softmax rows (t<128) live entirely in s-block 0, so keeping that
     block exact pins max rel err at the bf16 floor (~3e-3, validated offline).
     Diagonal 128x128 gets a multiplicative 0/1 triangle (DVE).
  5. AV: V[s,d] stationary per head, E^T moving; the two heads of a pair run
     CONCURRENTLY via col-tiling (output partitions 0-63 / 64-127 of one PSUM
     bank, tile_position (0,0)/(0,64) auto). Output is av^T[d-pair, t] --
     already in the y^T layout, no transposes. Softmax sums via 4 concurrent
     M=1 col-tiled matmuls (ones stationary) into partitions {0,32,64,96}.
  6. Host: y = (yt / sums) per head, transpose, concat.

Schedule: fused streaming pipeline per t-tile; next tile's projection closures
drain into the current tile's attention periods; AV lags scores by 2 s-blocks.
PSUM: scores 2x2 banks + av 2x1 + sums 1 + proj 1 = exactly 8 banks.
"""

import numpy as np

import concourse.mybir as mybir
import concourse.tile as tile
from concourse import bacc

B, T, C, H, D = 2, 2048, 1024, 16, 64
HPC = 4          # heads per core
NPAIR = 2        # head pairs per core
NCORES = 8
TT = 512         # t-tile
SB = 128         # s-block
NCH = C // 128   # contraction chunks
NTB = T // 128   # t-blocks (v-proj granularity)
F32 = mybir.dt.float32
BF16 = mybir.dt.bfloat16
F8 = mybir.dt.float8e4
I16 = mybir.dt.int16
WS = 32.0        # fp8 weight pre-scale (W*32 fits e4m3; drains divide it out)

# exp(x) ~= bitcast_bf16(int16(x * 128/ln2 + (127*128 - c))): Schraudolph in
# bf16 bits. Scores are in [-1, 1] so the int is ~16065..16435: no overflow,
# no sign issues. c calibrated offline; end-to-end error is insensitive to c
# and to round-vs-floor convert semantics (validated in numpy).
SCH_K = float(2.0 ** 7 / np.log(2.0))
SCH_B = float(127 * 128 - 5.0)

# exp engine assignment: s-block 0 goes to ScalarE for BOTH pairs (exact exp
# protects short softmax rows); for sb>0 the two pairs split across ScalarE
# and DVE (alternating by sb so each head sees a ~50/50 exact/approx mix and
# each S-buffer's release chain alternates engines -> no single-engine
# backlog stalls the scores matmuls on the PSUM WAR).


def build_nc():
    nj = T // TT
    nc = bacc.Bacc("TRN2", target_bir_lowering=False, debug=False)
    xt = nc.dram_tensor("xt", [C, T], BF16, kind="ExternalInput")
    xt8 = nc.dram_tensor("xt8", [C, T], F8, kind="ExternalInput")
    wq = nc.dram_tensor("wq", [C, HPC * D], F8, kind="ExternalInput")
    wk = nc.dram_tensor("wk", [C, HPC * D], F8, kind="ExternalInput")
    wv = nc.dram_tensor("wv", [C, HPC * D], BF16, kind="ExternalInput")
    yt = nc.dram_tensor("yt", [HPC * D, T], F32, kind="ExternalOutput")
    sm = nc.dram_tensor("sm", [nj, HPC, TT], F32, kind="ExternalOutput")

    with tile.TileContext(nc) as tc:
        with (
            tc.tile_pool(name="consts", bufs=1) as consts,
            tc.tile_pool(name="wpool", bufs=1) as wpool,
            tc.tile_pool(name="qkv", bufs=1) as qkv,
            tc.tile_pool(name="xtp", bufs=2) as xtp,
            tc.tile_pool(name="epool", bufs=6) as epool,
            tc.tile_pool(name="ysb", bufs=4) as ysbp,
            tc.tile_pool(name="smsb", bufs=2) as smsbp,
            tc.tile_pool(name="mixps", bufs=1, space="PSUM") as mixps,
            tc.tile_pool(name="spsum", bufs=2, space="PSUM") as spsum,
            tc.tile_pool(name="avps", bufs=2, space="PSUM") as avps,
            tc.tile_pool(name="smps", bufs=1, space="PSUM") as smps,
        ):
            ones1 = consts.tile([128, 1], BF16)
            nc.vector.memset(ones1, 1.0)
            # tri01[s, g, t_local] = 1 where t_local >= s else 0 (bf16 via a
            # one-time f32 build + cast; applied to the diagonal block of E).
            tri_f32 = consts.tile([128, 2, SB], F32)
            nc.gpsimd.memset(tri_f32, 1.0)
            for g in range(NPAIR):
                nc.gpsimd.affine_select(
                    out=tri_f32[:, g, :], in_=tri_f32[:, g, :],
                    compare_op=mybir.AluOpType.is_ge,
                    fill=0.0, base=0,
                    pattern=[[1, SB]], channel_multiplier=-1,
                )
            tri01 = consts.tile([128, 2, SB], BF16)
            nc.vector.tensor_copy(tri01, tri_f32)

            wq_sb = wpool.tile([128, NCH, HPC * D], F8, tag="wq")
            wk_sb = wpool.tile([128, NCH, HPC * D], F8, tag="wk")
            wv_sb = wpool.tile([128, NCH, HPC * D], BF16, tag="wv")
            w_sb = {"q": wq_sb, "k": wk_sb}

            QT = qkv.tile([128, NPAIR, T], BF16, tag="qt")
            KT = qkv.tile([128, NPAIR, T], BF16, tag="kt")
            V = qkv.tile([128, NTB, HPC * D], BF16, tag="v")

            def proj_closures(j):
                state = {}

                def do_load():
                    # fp8 copy first (q/k proj gates the pipeline fill);
                    # chunk-pair granularity so the first matmuls only wait
                    # for the first 2 contraction chunks, not the full tile.
                    x8r = xt8.rearrange("(k p) t -> p k t", p=128)
                    xt8_sb = xtp.tile([128, NCH, TT], F8, tag="xts8",
                                      name=f"xts8_{j}")
                    for kk in range(0, NCH, 2):
                        nc.sync.dma_start(
                            out=xt8_sb[:, kk:kk + 2, :],
                            in_=x8r[:, kk:kk + 2, j * TT:(j + 1) * TT])
                    xr = xt.rearrange("(k p) t -> p k t", p=128)
                    xt_sb = xtp.tile([128, NCH, TT], BF16, tag="xts",
                                     name=f"xts{j}")
                    for kk in range(0, NCH, 4):
                        nc.sync.dma_start(
                            out=xt_sb[:, kk:kk + 4, :],
                            in_=xr[:, kk:kk + 4, j * TT:(j + 1) * TT])
                    state["xt"] = xt_sb
                    state["xt8"] = xt8_sb

                def do_qk(name, g):
                    # fp8e4 DoubleRow: two contraction chunks per matmul
                    pp = mixps.tile([128, TT], F32, tag="mix",
                                    name=f"pp_{name}{g}_{j}")
                    for kk in range(0, NCH, 2):
                        nc.tensor.matmul(
                            pp,
                            lhsT=w_sb[name][:, kk:kk + 2, g * 128:(g + 1) * 128],
                            rhs=state["xt8"][:, kk:kk + 2, :],
                            start=(kk == 0), stop=(kk == NCH - 2),
                            skip_group_check=True,
                            perf_mode=mybir.MatmulPerfMode.DoubleRow,
                        )
                    dst = QT if name == "q" else KT
                    nc.vector.tensor_scalar(
                        dst[:, g, j * TT:(j + 1) * TT], pp,
                        1.0 / WS, None, mybir.AluOpType.mult)

                def do_v(tb2):
                    vp = mixps.tile([128, TT], F32, tag="mix",
                                    name=f"vp{tb2}_{j}")
                    for half in range(2):
                        tl = tb2 * 2 + half
                        for kk in range(NCH):
                            nc.tensor.matmul(
                                vp[:, half * 256:(half + 1) * 256],
                                lhsT=state["xt"][:, kk, tl * 128:(tl + 1) * 128],
                                rhs=wv_sb[:, kk, :],
                                start=(kk == 0), stop=(kk == NCH - 1),
                                skip_group_check=True,
                            )
                    gtb = j * 4 + tb2 * 2
                    nc.vector.tensor_copy(
                        V[:, gtb:gtb + 2, :],
                        vp.rearrange("p (a b) -> p a b", a=2))

                ops = [lambda: do_qk("q", 0), lambda: do_qk("q", 1),
                       lambda: do_qk("k", 0), lambda: do_qk("k", 1),
                       lambda: do_v(0), lambda: do_v(1)]
                return do_load, ops

            def emit_attention(j, pending):
                n_sb = 4 * (j + 1)
                av_ps = {g: avps.tile([128, TT], F32, tag="avps",
                                      name=f"av{j}_{g}")
                         for g in range(NPAIR)}
                sm_ps = smps.tile([128, TT], F32, tag="smps", name=f"smps{j}")
                eg = {}
                LAG = 2

                def emit_scores(sb):
                    off = max(0, (sb - 4 * j) * SB)
                    for g in range(NPAIR):
                        S = spsum.tile([128, 2, TT], F32, tag="spsum",
                                       name=f"s{j}_{sb}_{g}")
                        for hh in range(2):
                            hp = slice(hh * 64, (hh + 1) * 64)
                            nc.tensor.matmul(
                                S[:, hh, off:],
                                lhsT=KT[hp, g, sb * SB:(sb + 1) * SB],
                                rhs=QT[hp, g, j * TT + off:(j + 1) * TT],
                                start=True, stop=True,
                            )
                        e = epool.tile([128, 2, TT], BF16, tag="e",
                                       name=f"e{j}_{sb}_{g}")
                        if sb != 0 and g != (sb & 1):
                            nc.vector.tensor_scalar(
                                e.bitcast(I16)[:, :, off:],
                                S[:, :, off:],
                                SCH_K, SCH_B,
                                mybir.AluOpType.mult, mybir.AluOpType.add)
                        else:
                            nc.scalar.activation(
                                out=e[:, :, off:], in_=S[:, :, off:],
                                func=mybir.ActivationFunctionType.Exp)
                        if sb >= 4 * j:  # diagonal block: causal triangle
                            nc.vector.tensor_mul(
                                e[:, :, off:off + SB],
                                e[:, :, off:off + SB], tri01)
                        eg[(sb, g)] = e

                def emit_av(sb):
                    off = max(0, (sb - 4 * j) * SB)
                    es = {g: eg.pop((sb, g)) for g in range(NPAIR)}
                    for g in range(NPAIR):
                        for hh in range(2):
                            h = 2 * g + hh
                            nc.tensor.matmul(
                                av_ps[g][hh * 64:(hh + 1) * 64, off:],
                                lhsT=V[:, sb, h * 64:(h + 1) * 64],
                                rhs=es[g][:, hh, off:],
                                start=(sb == 0), stop=(sb == n_sb - 1),
                                skip_group_check=True,
                            )
                    for g in range(NPAIR):
                        for hh in range(2):
                            h = 2 * g + hh
                            nc.tensor.matmul(
                                sm_ps[32 * h:32 * h + 1, off:],
                                lhsT=ones1,
                                rhs=es[g][:, hh, off:],
                                start=(sb == 0), stop=(sb == n_sb - 1),
                                skip_group_check=True,
                                tile_position=(0, 32 * h),
                            )

                n_periods = n_sb + LAG
                n_pend = len(pending)
                popped = 0
                for sb in range(n_periods):
                    if sb < n_sb:
                        emit_scores(sb)
                    want = (n_pend * (sb + 1)) // n_periods
                    # the first pending items are the next xt load and the
                    # PREVIOUS tile's output drains; force them out in the
                    # first two periods -- a not-yet-drained av/sums PSUM
                    # bank would stall this tile's first AV matmul at the
                    # head of the PE queue, blocking everything behind it.
                    if sb == 0:
                        want = max(want, min(n_pend, 3))
                    elif sb == 1:
                        want = max(want, min(n_pend, 4))
                    while popped < want:
                        pending[popped]()
                        popped += 1
                    if sb >= LAG:
                        emit_av(sb - LAG)
                assert popped == n_pend

                # Output drains, returned as closures and emitted early in
                # the NEXT tile's attention (after its first exp) so the
                # ScalarE copies don't sit ahead of that tile's first exps
                # in the queue. They run on ScalarE (idle vs DVE; closer to
                # PSUM).
                def drain_y(g):
                    y_sb = ysbp.tile([128, TT], F32, tag="ysb",
                                     name=f"y{j}_{g}")
                    nc.scalar.copy(y_sb, av_ps[g])
                    nc.sync.dma_start(
                        out=yt[g * 128:(g + 1) * 128, j * TT:(j + 1) * TT],
                        in_=y_sb)

                def drain_sm():
                    # sums live on strided partitions {0,32,64,96}; engine
                    # APs must start on 32-aligned partitions, so compact
                    # onto partition 0 along the free dim before the DMA.
                    sm_sb = smsbp.tile([1, HPC, TT], F32, tag="smsb",
                                       name=f"sm{j}")
                    for h in range(HPC):
                        nc.scalar.copy(
                            sm_sb[:, h, :], sm_ps[32 * h:32 * h + 1, :])
                    nc.sync.dma_start(out=sm[j:j + 1], in_=sm_sb)

                return [lambda: drain_y(0), lambda: drain_y(1), drain_sm]

            ld0, ops0 = proj_closures(0)
            ld0()
            nc.sync.dma_start(
                out=wq_sb, in_=wq.rearrange("(k p) d -> p k d", p=128))
            sm_ps_warm = smps.tile([128, TT], F32, tag="smps", name="smwarm")
            # PE warm-up: ~5us of dependency-free tiny matmuls during the
            # DMA fill so the HAM clock-gate is at 8/8 when the first
            # projection matmul issues (cold K=4/8 costs 2x for ~10us).
            # Writes scratch into the sums bank; attention later overwrites
            # it via start=True.
            for _ in range(220):
                nc.tensor.matmul(sm_ps_warm[0:1, 0:1], lhsT=ones1,
                                 rhs=ones1, start=True, stop=True,
                                 skip_group_check=True)
            # q-proj right after the wq DMA so its weight loads wait only on
            # wq (not on wk/wv); then interleave the remaining W DMAs.
            ops0[0]()
            ops0[1]()
            nc.sync.dma_start(
                out=wk_sb, in_=wk.rearrange("(k p) d -> p k d", p=128))
            ops0[2]()
            ops0[3]()
            nc.sync.dma_start(
                out=wv_sb, in_=wv.rearrange("(k p) d -> p k d", p=128))
            ops0[4]()
            ops0[5]()
            drains = []
            for j in range(nj):
                if j + 1 < nj:
                    ldn, opsn = proj_closures(j + 1)
                    nxt = [ldn] + drains + opsn
                else:
                    nxt = list(drains)
                drains = emit_attention(j, nxt)
            for dr in drains:
                dr()

    nc.compile()
    return nc


_CACHE = {}


def _get_runner():
    if "run" in _CACHE:
        return _CACHE["run"]

    import jax
    from jax.experimental.shard_map import shard_map
    from jax.sharding import Mesh, PartitionSpec
    from concourse import bass2jax
    from concourse.bass2jax import _bass_exec_p, install_neuronx_cc_hook

    nc = build_nc()
    install_neuronx_cc_hook()

    partition_name = (nc.partition_id_tensor.name
                      if nc.partition_id_tensor else None)
    in_names, out_names, out_avals, zero_outs = [], [], [], []
    for alloc in nc.m.functions[0].allocations:
        if not isinstance(alloc, mybir.MemoryLocationSet):
            continue
        name = alloc.memorylocations[0].name
        if alloc.kind == "ExternalInput":
            if name != partition_name:
                in_names.append(name)
        elif alloc.kind == "ExternalOutput":
            out_names.append(name)
            shape = tuple(alloc.tensor_shape)
            dtype = mybir.dt.np(alloc.dtype)
            out_avals.append(jax.core.ShapedArray(shape, dtype))
            zero_outs.append(np.zeros(shape, dtype))
    n_params = len(in_names)
    n_outs = len(out_avals)
    all_names = in_names + out_names
    if partition_name is not None:
        all_names = all_names + [partition_name]
    donate = tuple(range(n_params, n_params + n_outs))

    def _body(*args):
        operands = list(args)
        if partition_name is not None:
            operands.append(bass2jax.partition_id_tensor())
        outs = _bass_exec_p.bind(
            *operands,
            out_avals=tuple(out_avals),
            in_names=tuple(all_names),
            out_names=tuple(out_names),
            lowering_input_output_aliases=(),
            sim_require_finite=True,
            sim_require_nnan=True,
            nc=nc,
        )
        return tuple(outs)

    devices = jax.devices()[:NCORES]
    mesh = Mesh(np.asarray(devices), ("core",))
    in_specs = (PartitionSpec("core"),) * (n_params + n_outs)
    out_specs = (PartitionSpec("core"),) * n_outs
    sharded = jax.jit(
        shard_map(_body, mesh=mesh, in_specs=in_specs, out_specs=out_specs,
                  check_rep=False),
        donate_argnums=donate, keep_unused=True,
    )

    runner = {
        "nc": nc,
        "all_names": all_names,
        "sharded": sharded,
        "in_names": in_names,
        "out_names": out_names,
        "out_avals": out_avals,
        "zero_outs": zero_outs,
    }
    _CACHE["run"] = runner
    return runner


def _shard_inputs(x, Wq, Wk, Wv):
    """Per-core input dicts. Host-side layout prep only."""
    bf = mybir.dt.np(BF16)
    f8 = mybir.dt.np(F8)
    scale = float(C) ** -0.5
    maps = []
    for c in range(NCORES):
        b, hg = divmod(c, 4)
        hs = list(range(HPC * hg, HPC * hg + HPC))
        xtb = np.ascontiguousarray(np.transpose(x[b]))  # [C, T]
        wq2 = np.ascontiguousarray(
            (np.concatenate([Wq[h] for h in hs], axis=1)
             * (scale * WS)).astype(f8))
        wk2 = np.ascontiguousarray(
            (np.concatenate([Wk[h] for h in hs], axis=1) * WS).astype(f8))
        wv2 = np.ascontiguousarray(
            np.concatenate([Wv[h] for h in hs], axis=1).astype(bf))
        maps.append({"xt": xtb.astype(bf), "xt8": xtb.astype(f8),
                     "wq": wq2, "wk": wk2, "wv": wv2})
    return maps


def run_sharded(in_maps):
    """Run the 8-core NEFF once; returns list of per-core output dicts."""
    r = _get_runner()
    concat_in = [
        np.concatenate([in_maps[c][name] for c in range(NCORES)], axis=0)
        for name in r["in_names"]
    ]
    concat_zeros = [
        np.zeros((NCORES * z.shape[0], *z.shape[1:]), z.dtype)
        for z in r["zero_outs"]
    ]
    out_arrs = r["sharded"](*concat_in, *concat_zeros)
    return [
        {
            name: np.asarray(out_arrs[i]).reshape(
                NCORES, *r["out_avals"][i].shape)[c]
            for i, name in enumerate(r["out_names"])
        }
        for c in range(NCORES)
    ]


def kernel(x, Wq, Wk, Wv):
    x = np.asarray(x, dtype=np.float32)
    Wq = np.asarray(Wq, dtype=np.float32)
    Wk = np.asarray(Wk, dtype=np.float32)
    Wv = np.asarray(Wv, dtype=np.float32)
    in_maps = _shard_inputs(x, Wq, Wk, Wv)
    results = run_sharded(in_maps)
    outs = []
    for b in range(B):
        parts = []
        for hg in range(4):
            r = results[b * 4 + hg]
            ytc = np.asarray(r["yt"], dtype=np.float32)   # [256, T]
            smc = np.asarray(r["sm"], dtype=np.float32)   # [nj, 4, TT]
            smc = smc.transpose(1, 0, 2).reshape(HPC, T)  # [4, T]
            yn = ytc.reshape(HPC, D, T) / smc[:, None, :]
            parts.append(yn.reshape(HPC * D, T).T)        # [T, 256]
        outs.append(np.concatenate(parts, axis=1))        # [T, 1024]
    return np.ascontiguousarray(np.stack(outs)).astype(np.float32)


# revision 25
# speedup vs baseline: 1.2843x; 1.0391x over previous
"""Multi-head causal attention (B=2, T=2048, C=1024, H=16, D=64) on 8 TRN2 cores.

Sharding: core c = (batch b = c//4, head-group hg = c%4): 4 heads of one batch
per core (halves x DMA vs replicating both batches). Host concatenates heads /
batches and normalizes (divide by softmax sums) + transposes on the way out.

Per-core dataflow, all matmuls bf16 x bf16 -> f32 PSUM (fast FWL weight loads;
LDWEIGHTS hides under the previous matmul, unlike f32r self-loading):
  1. q/k projections: W chunk stationary [128c, 128hd], x^T moving [128c, 512t]
     -> Q^T/K^T [hd, t] bf16, head pair g on partitions (h even: 0-63,
     h odd: 64-127). Scale 1/sqrt(C) folded into Wq on host.
  2. v projection TRANSPOSED: x^T chunk stationary [128c, 128t], Wv moving
     [128c, 256hd] -> V[t, hd] directly (no PE transposes at all).
  3. Scores S^T[s, 2, t] per (s-block, pair): K^T stationary, Q^T moving; the
     two heads of a pair run CONCURRENTLY via row-tiling (contraction d=64 ->
     tile_position (0,0)/(64,0) auto-derived from base partitions). Columns
     below the causal diagonal never computed (off trim).
  4. exp: s-block 0 (plus a tunable share) on ScalarE (exact); the rest on DVE
     via a 1-op Schraudolph: E_bf16bits = int16(S * 128/ln2 + (127*128-c)).
     Short softmax rows (t<128) live entirely in s-block 0, so keeping that
     block exact pins max rel err at the bf16 floor (~3e-3, validated offline).
     Diagonal 128x128 gets a multiplicative 0/1 triangle (DVE).
  5. AV: V[s,d] stationary per head, E^T moving; the two heads of a pair run
     CONCURRENTLY via col-tiling (output partitions 0-63 / 64-127 of one PSUM
     bank, tile_position (0,0)/(0,64) auto). Output is av^T[d-pair, t] --
     already in the y^T layout, no transposes. Softmax sums via 4 concurrent
     M=1 col-tiled matmuls (ones stationary) into partitions {0,32,64,96}.
  6. Host: y = (yt / sums) per head, transpose, concat.

Schedule: fused streaming pipeline per t-tile; next tile's projection closures
drain into the current tile's attention periods; AV lags scores by 2 s-blocks.
PSUM: scores 2x2 banks + av 2x1 + sums 1 + proj 1 = exactly 8 banks.
"""

import numpy as np

import concourse.mybir as mybir
import concourse.tile as tile
from concourse import bacc

B, T, C, H, D = 2, 2048, 1024, 16, 64
HPC = 4          # heads per core
NPAIR = 2        # head pairs per core
NCORES = 8
TT = 512         # t-tile
SB = 128         # s-block
NCH = C // 128   # contraction chunks
NTB = T // 128   # t-blocks (v-proj granularity)
F32 = mybir.dt.float32
BF16 = mybir.dt.bfloat16
F8 = mybir.dt.float8e4
I16 = mybir.dt.int16
WS = 32.0        # fp8 weight pre-scale (W*32 fits e4m3; drains divide it out)

# exp(x) ~= bitcast_bf16(int16(x * 128/ln2 + (127*128 - c))): Schraudolph in
# bf16 bits. Scores are in [-1, 1] so the int is ~16065..16435: no overflow,
# no sign issues. c calibrated offline; end-to-end error is insensitive to c
# and to round-vs-floor convert semantics (validated in numpy).
SCH_K = float(2.0 ** 7 / np.log(2.0))
SCH_B = float(127 * 128 - 5.0)

# exp engine assignment: s-block 0 goes to ScalarE for BOTH pairs (exact exp
# protects short softmax rows); for sb>0 the two pairs split across ScalarE
# and DVE (alternating by sb so each head sees a ~50/50 exact/approx mix and
# each S-buffer's release chain alternates engines -> no single-engine
# backlog stalls the scores matmuls on the PSUM WAR).


def build_nc():
    nj = T // TT
    nc = bacc.Bacc("TRN2", target_bir_lowering=False, debug=False)
    xt = nc.dram_tensor("xt", [C, T], BF16, kind="ExternalInput")
    xt8 = nc.dram_tensor("xt8", [C, T], F8, kind="ExternalInput")
    wq = nc.dram_tensor("wq", [C, HPC * D], F8, kind="ExternalInput")
    wk = nc.dram_tensor("wk", [C, HPC * D], F8, kind="ExternalInput")
    wv = nc.dram_tensor("wv", [C, HPC * D], BF16, kind="ExternalInput")
    yt = nc.dram_tensor("yt", [HPC * D, T], F32, kind="ExternalOutput")
    sm = nc.dram_tensor("sm", [nj, HPC, TT], F32, kind="ExternalOutput")

    with tile.TileContext(nc) as tc:
        with (
            tc.tile_pool(name="consts", bufs=1) as consts,
            tc.tile_pool(name="wpool", bufs=1) as wpool,
            tc.tile_pool(name="qkv", bufs=1) as qkv,
            tc.tile_pool(name="xtp", bufs=2) as xtp,
            tc.tile_pool(name="epool", bufs=8) as epool,
            tc.tile_pool(name="ysb", bufs=4) as ysbp,
            tc.tile_pool(name="smsb", bufs=2) as smsbp,
            tc.tile_pool(name="mixps", bufs=1, space="PSUM") as mixps,
            tc.tile_pool(name="spsum", bufs=2, space="PSUM") as spsum,
            tc.tile_pool(name="avps", bufs=2, space="PSUM") as avps,
            tc.tile_pool(name="smps", bufs=1, space="PSUM") as smps,
        ):
            ones1 = consts.tile([128, 1], BF16)
            nc.vector.memset(ones1, 1.0)
            # tri01[s, g, t_local] = 1 where t_local >= s else 0 (bf16 via a
            # one-time f32 build + cast; applied to the diagonal block of E).
            tri_f32 = consts.tile([128, 2, SB], F32)
            nc.gpsimd.memset(tri_f32, 1.0)
            for g in range(NPAIR):
                nc.gpsimd.affine_select(
                    out=tri_f32[:, g, :], in_=tri_f32[:, g, :],
                    compare_op=mybir.AluOpType.is_ge,
                    fill=0.0, base=0,
                    pattern=[[1, SB]], channel_multiplier=-1,
                )
            tri01 = consts.tile([128, 2, SB], BF16)
            nc.vector.tensor_copy(tri01, tri_f32)

            wq_sb = wpool.tile([128, NCH, HPC * D], F8, tag="wq")
            wk_sb = wpool.tile([128, NCH, HPC * D], F8, tag="wk")
            wv_sb = wpool.tile([128, NCH, HPC * D], BF16, tag="wv")
            w_sb = {"q": wq_sb, "k": wk_sb}

            QT = qkv.tile([128, NPAIR, T], BF16, tag="qt")
            KT = qkv.tile([128, NPAIR, T], BF16, tag="kt")
            V = qkv.tile([128, NTB, HPC * D], BF16, tag="v")

            def proj_closures(j):
                state = {}

                def do_load():
                    # fp8 copy first (q/k proj gates the pipeline fill);
                    # chunk-pair granularity so the first matmuls only wait
                    # for the first 2 contraction chunks, not the full tile.
                    x8r = xt8.rearrange("(k p) t -> p k t", p=128)
                    xt8_sb = xtp.tile([128, NCH, TT], F8, tag="xts8",
                                      name=f"xts8_{j}")
                    for kk in range(0, NCH, 2):
                        nc.sync.dma_start(
                            out=xt8_sb[:, kk:kk + 2, :],
                            in_=x8r[:, kk:kk + 2, j * TT:(j + 1) * TT])
                    xr = xt.rearrange("(k p) t -> p k t", p=128)
                    xt_sb = xtp.tile([128, NCH, TT], BF16, tag="xts",
                                     name=f"xts{j}")
                    for kk in range(0, NCH, 4):
                        nc.sync.dma_start(
                            out=xt_sb[:, kk:kk + 4, :],
                            in_=xr[:, kk:kk + 4, j * TT:(j + 1) * TT])
                    state["xt"] = xt_sb
                    state["xt8"] = xt8_sb

                def do_qk(name, g):
                    # fp8e4 DoubleRow: two contraction chunks per matmul
                    pp = mixps.tile([128, TT], F32, tag="mix",
                                    name=f"pp_{name}{g}_{j}")
                    for kk in range(0, NCH, 2):
                        nc.tensor.matmul(
                            pp,
                            lhsT=w_sb[name][:, kk:kk + 2, g * 128:(g + 1) * 128],
                            rhs=state["xt8"][:, kk:kk + 2, :],
                            start=(kk == 0), stop=(kk == NCH - 2),
                            skip_group_check=True,
                            perf_mode=mybir.MatmulPerfMode.DoubleRow,
                        )
                    dst = QT if name == "q" else KT
                    nc.vector.tensor_scalar(
                        dst[:, g, j * TT:(j + 1) * TT], pp,
                        1.0 / WS, None, mybir.AluOpType.mult)

                def do_v(tb2):
                    vp = mixps.tile([128, TT], F32, tag="mix",
                                    name=f"vp{tb2}_{j}")
                    for half in range(2):
                        tl = tb2 * 2 + half
                        for kk in range(NCH):
                            nc.tensor.matmul(
                                vp[:, half * 256:(half + 1) * 256],
                                lhsT=state["xt"][:, kk, tl * 128:(tl + 1) * 128],
                                rhs=wv_sb[:, kk, :],
                                start=(kk == 0), stop=(kk == NCH - 1),
                                skip_group_check=True,
                            )
                    gtb = j * 4 + tb2 * 2
                    nc.vector.tensor_copy(
                        V[:, gtb:gtb + 2, :],
                        vp.rearrange("p (a b) -> p a b", a=2))

                ops = [lambda: do_qk("q", 0), lambda: do_qk("q", 1),
                       lambda: do_qk("k", 0), lambda: do_qk("k", 1),
                       lambda: do_v(0), lambda: do_v(1)]
                return do_load, ops

            def emit_attention(j, pending):
                n_sb = 4 * (j + 1)
                av_ps = {g: avps.tile([128, TT], F32, tag="avps",
                                      name=f"av{j}_{g}")
                         for g in range(NPAIR)}
                sm_ps = smps.tile([128, TT], F32, tag="smps", name=f"smps{j}")
                eg = {}
                LAG = 2

                def emit_scores(sb):
                    off = max(0, (sb - 4 * j) * SB)
                    for g in range(NPAIR):
                        S = spsum.tile([128, 2, TT], F32, tag="spsum",
                                       name=f"s{j}_{sb}_{g}")
                        for hh in range(2):
                            hp = slice(hh * 64, (hh + 1) * 64)
                            nc.tensor.matmul(
                                S[:, hh, off:],
                                lhsT=KT[hp, g, sb * SB:(sb + 1) * SB],
                                rhs=QT[hp, g, j * TT + off:(j + 1) * TT],
                                start=True, stop=True,
                            )
                        e = epool.tile([128, 2, TT], BF16, tag="e",
                                       name=f"e{j}_{sb}_{g}")
                        if sb != 0 and g != (sb & 1):
                            nc.vector.tensor_scalar(
                                e.bitcast(I16)[:, :, off:],
                                S[:, :, off:],
                                SCH_K, SCH_B,
                                mybir.AluOpType.mult, mybir.AluOpType.add)
                        else:
                            nc.scalar.activation(
                                out=e[:, :, off:], in_=S[:, :, off:],
                                func=mybir.ActivationFunctionType.Exp)
                        if sb >= 4 * j:  # diagonal block: causal triangle
                            nc.vector.tensor_mul(
                                e[:, :, off:off + SB],
                                e[:, :, off:off + SB], tri01)
                        eg[(sb, g)] = e

                def emit_av(sb):
                    off = max(0, (sb - 4 * j) * SB)
                    es = {g: eg.pop((sb, g)) for g in range(NPAIR)}
                    for g in range(NPAIR):
                        for hh in range(2):
                            h = 2 * g + hh
                            nc.tensor.matmul(
                                av_ps[g][hh * 64:(hh + 1) * 64, off:],
                                lhsT=V[:, sb, h * 64:(h + 1) * 64],
                                rhs=es[g][:, hh, off:],
                                start=(sb == 0), stop=(sb == n_sb - 1),
                                skip_group_check=True,
                            )
                    for g in range(NPAIR):
                        for hh in range(2):
                            h = 2 * g + hh
                            nc.tensor.matmul(
                                sm_ps[32 * h:32 * h + 1, off:],
                                lhsT=ones1,
                                rhs=es[g][:, hh, off:],
                                start=(sb == 0), stop=(sb == n_sb - 1),
                                skip_group_check=True,
                                tile_position=(0, 32 * h),
                            )

                n_periods = n_sb + LAG
                n_pend = len(pending)
                popped = 0
                # Drain all pending work BEFORE the tail periods: anything
                # queued between this tile's last scores and its final AV
                # matmuls delays the av stop -> delays the output drains ->
                # (via the ScalarE queue) stalls the next tile's exps and
                # lets the HAM clock-gate re-throttle the PE.
                ramp = max(n_sb - LAG - 1, 4)
                for sb in range(n_periods):
                    if sb < n_sb:
                        emit_scores(sb)
                    want = min(n_pend, (n_pend * (sb + 1)) // ramp)
                    # the first pending items are the next xt load and the
                    # PREVIOUS tile's output drains; force them out in the
                    # first two periods -- a not-yet-drained av/sums PSUM
                    # bank would stall this tile's first AV matmul at the
                    # head of the PE queue, blocking everything behind it.
                    if sb == 0:
                        want = max(want, min(n_pend, 3))
                    elif sb == 1:
                        want = max(want, min(n_pend, 4))
                    while popped < want:
                        pending[popped]()
                        popped += 1
                    if sb >= LAG:
                        emit_av(sb - LAG)
                assert popped == n_pend

                # Output drains, returned as closures and emitted early in
                # the NEXT tile's attention (after its first exp) so the
                # ScalarE copies don't sit ahead of that tile's first exps
                # in the queue. They run on ScalarE (idle vs DVE; closer to
                # PSUM).
                last = j == nj - 1

                def drain_y(g):
                    y_sb = ysbp.tile([128, TT], F32, tag="ysb",
                                     name=f"y{j}_{g}")
                    # final tile: split across engines to shorten the tail
                    if last and g == 1:
                        nc.vector.tensor_copy(y_sb, av_ps[g])
                    else:
                        nc.scalar.copy(y_sb, av_ps[g])
                    nc.sync.dma_start(
                        out=yt[g * 128:(g + 1) * 128, j * TT:(j + 1) * TT],
                        in_=y_sb)

                def drain_sm():
                    # sums live on strided partitions {0,32,64,96}; engine
                    # APs must start on 32-aligned partitions, so compact
                    # onto partition 0 along the free dim before the DMA.
                    sm_sb = smsbp.tile([1, HPC, TT], F32, tag="smsb",
                                       name=f"sm{j}")
                    for h in range(HPC):
                        if last and h % 2:
                            nc.vector.tensor_copy(
                                sm_sb[:, h, :], sm_ps[32 * h:32 * h + 1, :])
                        else:
                            nc.scalar.copy(
                                sm_sb[:, h, :], sm_ps[32 * h:32 * h + 1, :])
                    nc.sync.dma_start(out=sm[j:j + 1], in_=sm_sb)

                return [lambda: drain_y(0), lambda: drain_y(1), drain_sm]

            ld0, ops0 = proj_closures(0)
            ld0()
            nc.sync.dma_start(
                out=wq_sb, in_=wq.rearrange("(k p) d -> p k d", p=128))
            sm_ps_warm = smps.tile([128, TT], F32, tag="smps", name="smwarm")
            # PE warm-up: ~5us of dependency-free tiny matmuls during the
            # DMA fill so the HAM clock-gate is at 8/8 when the first
            # projection matmul issues (cold K=4/8 costs 2x for ~10us).
            # Writes scratch into the sums bank; attention later overwrites
            # it via start=True.
            for _ in range(120):
                nc.tensor.matmul(sm_ps_warm[0:1, 0:1], lhsT=ones1,
                                 rhs=ones1, start=True, stop=True,
                                 skip_group_check=True)
            # q-proj right after the wq DMA so its weight loads wait only on
            # wq (not on wk/wv); then interleave the remaining W DMAs.
            ops0[0]()
            ops0[1]()
            nc.sync.dma_start(
                out=wk_sb, in_=wk.rearrange("(k p) d -> p k d", p=128))
            ops0[2]()
            ops0[3]()
            nc.sync.dma_start(
                out=wv_sb, in_=wv.rearrange("(k p) d -> p k d", p=128))
            ops0[4]()
            ops0[5]()
            drains = []
            for j in range(nj):
                if j + 1 < nj:
                    ldn, opsn = proj_closures(j + 1)
                    nxt = [ldn] + drains + opsn
                else:
                    nxt = list(drains)
                drains = emit_attention(j, nxt)
            for dr in drains:
                dr()

    nc.compile()
    return nc


_CACHE = {}


def _get_runner():
    if "run" in _CACHE:
        return _CACHE["run"]

    import jax
    from jax.experimental.shard_map import shard_map
    from jax.sharding import Mesh, PartitionSpec
    from concourse import bass2jax
    from concourse.bass2jax import _bass_exec_p, install_neuronx_cc_hook

    nc = build_nc()
    install_neuronx_cc_hook()

    partition_name = (nc.partition_id_tensor.name
                      if nc.partition_id_tensor else None)
    in_names, out_names, out_avals, zero_outs = [], [], [], []
    for alloc in nc.m.functions[0].allocations:
        if not isinstance(alloc, mybir.MemoryLocationSet):
            continue
        name = alloc.memorylocations[0].name
        if alloc.kind == "ExternalInput":
            if name != partition_name:
                in_names.append(name)
        elif alloc.kind == "ExternalOutput":
            out_names.append(name)
            shape = tuple(alloc.tensor_shape)
            dtype = mybir.dt.np(alloc.dtype)
            out_avals.append(jax.core.ShapedArray(shape, dtype))
            zero_outs.append(np.zeros(shape, dtype))
    n_params = len(in_names)
    n_outs = len(out_avals)
    all_names = in_names + out_names
    if partition_name is not None:
        all_names = all_names + [partition_name]
    donate = tuple(range(n_params, n_params + n_outs))

    def _body(*args):
        operands = list(args)
        if partition_name is not None:
            operands.append(bass2jax.partition_id_tensor())
        outs = _bass_exec_p.bind(
            *operands,
            out_avals=tuple(out_avals),
            in_names=tuple(all_names),
            out_names=tuple(out_names),
            lowering_input_output_aliases=(),
            sim_require_finite=True,
            sim_require_nnan=True,
            nc=nc,
        )
        return tuple(outs)

    devices = jax.devices()[:NCORES]
    mesh = Mesh(np.asarray(devices), ("core",))
    in_specs = (PartitionSpec("core"),) * (n_params + n_outs)
    out_specs = (PartitionSpec("core"),) * n_outs
    sharded = jax.jit(
        shard_map(_body, mesh=mesh, in_specs=in_specs, out_specs=out_specs,
                  check_rep=False),
        donate_argnums=donate, keep_unused=True,
    )

    runner = {
        "nc": nc,
        "all_names": all_names,
        "sharded": sharded,
        "in_names": in_names,
        "out_names": out_names,
        "out_avals": out_avals,
        "zero_outs": zero_outs,
    }
    _CACHE["run"] = runner
    return runner


def _shard_inputs(x, Wq, Wk, Wv):
    """Per-core input dicts. Host-side layout prep only."""
    bf = mybir.dt.np(BF16)
    f8 = mybir.dt.np(F8)
    scale = float(C) ** -0.5
    maps = []
    for c in range(NCORES):
        b, hg = divmod(c, 4)
        hs = list(range(HPC * hg, HPC * hg + HPC))
        xtb = np.ascontiguousarray(np.transpose(x[b]))  # [C, T]
        wq2 = np.ascontiguousarray(
            (np.concatenate([Wq[h] for h in hs], axis=1)
             * (scale * WS)).astype(f8))
        wk2 = np.ascontiguousarray(
            (np.concatenate([Wk[h] for h in hs], axis=1) * WS).astype(f8))
        wv2 = np.ascontiguousarray(
            np.concatenate([Wv[h] for h in hs], axis=1).astype(bf))
        maps.append({"xt": xtb.astype(bf), "xt8": xtb.astype(f8),
                     "wq": wq2, "wk": wk2, "wv": wv2})
    return maps


def run_sharded(in_maps):
    """Run the 8-core NEFF once; returns list of per-core output dicts."""
    r = _get_runner()
    concat_in = [
        np.concatenate([in_maps[c][name] for c in range(NCORES)], axis=0)
        for name in r["in_names"]
    ]
    concat_zeros = [
        np.zeros((NCORES * z.shape[0], *z.shape[1:]), z.dtype)
        for z in r["zero_outs"]
    ]
    out_arrs = r["sharded"](*concat_in, *concat_zeros)
    return [
        {
            name: np.asarray(out_arrs[i]).reshape(
                NCORES, *r["out_avals"][i].shape)[c]
            for i, name in enumerate(r["out_names"])
        }
        for c in range(NCORES)
    ]


def kernel(x, Wq, Wk, Wv):
    x = np.asarray(x, dtype=np.float32)
    Wq = np.asarray(Wq, dtype=np.float32)
    Wk = np.asarray(Wk, dtype=np.float32)
    Wv = np.asarray(Wv, dtype=np.float32)
    in_maps = _shard_inputs(x, Wq, Wk, Wv)
    results = run_sharded(in_maps)
    outs = []
    for b in range(B):
        parts = []
        for hg in range(4):
            r = results[b * 4 + hg]
            ytc = np.asarray(r["yt"], dtype=np.float32)   # [256, T]
            smc = np.asarray(r["sm"], dtype=np.float32)   # [nj, 4, TT]
            smc = smc.transpose(1, 0, 2).reshape(HPC, T)  # [4, T]
            yn = ytc.reshape(HPC, D, T) / smc[:, None, :]
            parts.append(yn.reshape(HPC * D, T).T)        # [T, 256]
        outs.append(np.concatenate(parts, axis=1))        # [T, 1024]
    return np.ascontiguousarray(np.stack(outs)).astype(np.float32)


# revision 30
# speedup vs baseline: 1.2945x; 1.0080x over previous
"""Multi-head causal attention (B=2, T=2048, C=1024, H=16, D=64) on 8 TRN2 cores.

Sharding: core c = (batch b = c//4, head-group hg = c%4): 4 heads of one batch
per core (halves x DMA vs replicating both batches). Host concatenates heads /
batches and normalizes (divide by softmax sums) + transposes on the way out.

Per-core dataflow, all matmuls bf16 x bf16 -> f32 PSUM (fast FWL weight loads;
LDWEIGHTS hides under the previous matmul, unlike f32r self-loading):
  1. q/k projections: W chunk stationary [128c, 128hd], x^T moving [128c, 512t]
     -> Q^T/K^T [hd, t] bf16, head pair g on partitions (h even: 0-63,
     h odd: 64-127). Scale 1/sqrt(C) folded into Wq on host.
  2. v projection TRANSPOSED: x^T chunk stationary [128c, 128t], Wv moving
     [128c, 256hd] -> V[t, hd] directly (no PE transposes at all).
  3. Scores S^T[s, 2, t] per (s-block, pair): K^T stationary, Q^T moving; the
     two heads of a pair run CONCURRENTLY via row-tiling (contraction d=64 ->
     tile_position (0,0)/(64,0) auto-derived from base partitions). Columns
     below the causal diagonal never computed (off trim).
  4. exp: s-block 0 (plus a tunable share) on ScalarE (exact); the rest on DVE
     via a 1-op Schraudolph: E_bf16bits = int16(S * 128/ln2 + (127*128-c)).
     Short softmax rows (t<128) live entirely in s-block 0, so keeping that
     block exact pins max rel err at the bf16 floor (~3e-3, validated offline).
     Diagonal 128x128 gets a multiplicative 0/1 triangle (DVE).
  5. AV: V[s,d] stationary per head, E^T moving; the two heads of a pair run
     CONCURRENTLY via col-tiling (output partitions 0-63 / 64-127 of one PSUM
     bank, tile_position (0,0)/(0,64) auto). Output is av^T[d-pair, t] --
     already in the y^T layout, no transposes. Softmax sums via 4 concurrent
     M=1 col-tiled matmuls (ones stationary) into partitions {0,32,64,96}.
  6. Host: y = (yt / sums) per head, transpose, concat.

Schedule: fused streaming pipeline per t-tile; next tile's projection closures
drain into the current tile's attention periods; AV lags scores by 2 s-blocks.
PSUM: scores 2x2 banks + av 2x1 + sums 1 + proj 1 = exactly 8 banks.
"""

import numpy as np

import concourse.mybir as mybir
import concourse.tile as tile
from concourse import bacc

B, T, C, H, D = 2, 2048, 1024, 16, 64
HPC = 4          # heads per core
NPAIR = 2        # head pairs per core
NCORES = 8
TT = 512         # t-tile
SB = 128         # s-block
NCH = C // 128   # contraction chunks
NTB = T // 128   # t-blocks (v-proj granularity)
F32 = mybir.dt.float32
BF16 = mybir.dt.bfloat16
F8 = mybir.dt.float8e4
I16 = mybir.dt.int16
WS = 32.0        # fp8 weight pre-scale (W*32 fits e4m3; drains divide it out)

# exp(x) ~= bitcast_bf16(int16(x * 128/ln2 + (127*128 - c))): Schraudolph in
# bf16 bits. Scores are in [-1, 1] so the int is ~16065..16435: no overflow,
# no sign issues. c calibrated offline; end-to-end error is insensitive to c
# and to round-vs-floor convert semantics (validated in numpy).
SCH_K = float(2.0 ** 7 / np.log(2.0))
SCH_B = float(127 * 128 - 5.0)

# exp engine assignment: s-block 0 goes to ScalarE for BOTH pairs (exact exp
# protects short softmax rows); for sb>0 the two pairs split across ScalarE
# and DVE (alternating by sb so each head sees a ~50/50 exact/approx mix and
# each S-buffer's release chain alternates engines -> no single-engine
# backlog stalls the scores matmuls on the PSUM WAR).


def build_nc():
    nj = T // TT
    nc = bacc.Bacc("TRN2", target_bir_lowering=False, debug=False)
    xt = nc.dram_tensor("xt", [C, T], BF16, kind="ExternalInput")
    xt8 = nc.dram_tensor("xt8", [C, T], F8, kind="ExternalInput")
    wq = nc.dram_tensor("wq", [C, HPC * D], F8, kind="ExternalInput")
    wk = nc.dram_tensor("wk", [C, HPC * D], F8, kind="ExternalInput")
    wv = nc.dram_tensor("wv", [C, HPC * D], BF16, kind="ExternalInput")
    yt = nc.dram_tensor("yt", [HPC * D, T], F32, kind="ExternalOutput")
    sm = nc.dram_tensor("sm", [nj, HPC, TT], F32, kind="ExternalOutput")

    with tile.TileContext(nc) as tc:
        with (
            tc.tile_pool(name="consts", bufs=1) as consts,
            tc.tile_pool(name="wpool", bufs=1) as wpool,
            tc.tile_pool(name="qkv", bufs=1) as qkv,
            tc.tile_pool(name="xtp", bufs=2) as xtp,
            tc.tile_pool(name="epool", bufs=8) as epool,
            tc.tile_pool(name="ysb", bufs=4) as ysbp,
            tc.tile_pool(name="smsb", bufs=2) as smsbp,
            tc.tile_pool(name="mixps", bufs=1, space="PSUM") as mixps,
            tc.tile_pool(name="spsum", bufs=2, space="PSUM") as spsum,
            tc.tile_pool(name="avps", bufs=2, space="PSUM") as avps,
            tc.tile_pool(name="smps", bufs=1, space="PSUM") as smps,
        ):
            ones1 = consts.tile([128, 1], BF16)
            nc.vector.memset(ones1, 1.0)
            # tri01[s, g, t_local] = 1 where t_local >= s else 0 (bf16 via a
            # one-time f32 build + cast; applied to the diagonal block of E).
            tri_f32 = consts.tile([128, 2, SB], F32)
            nc.gpsimd.memset(tri_f32, 1.0)
            for g in range(NPAIR):
                nc.gpsimd.affine_select(
                    out=tri_f32[:, g, :], in_=tri_f32[:, g, :],
                    compare_op=mybir.AluOpType.is_ge,
                    fill=0.0, base=0,
                    pattern=[[1, SB]], channel_multiplier=-1,
                )
            tri01 = consts.tile([128, 2, SB], BF16)
            nc.vector.tensor_copy(tri01, tri_f32)

            wq_sb = wpool.tile([128, NCH, HPC * D], F8, tag="wq")
            wk_sb = wpool.tile([128, NCH, HPC * D], F8, tag="wk")
            wv_sb = wpool.tile([128, NCH, HPC * D], BF16, tag="wv")
            w_sb = {"q": wq_sb, "k": wk_sb}

            QT = qkv.tile([128, NPAIR, T], BF16, tag="qt")
            KT = qkv.tile([128, NPAIR, T], BF16, tag="kt")
            V = qkv.tile([128, NTB, HPC * D], BF16, tag="v")

            def proj_closures(j):
                state = {}

                def do_load8():
                    # fp8 copy first (q/k proj gates the pipeline fill);
                    # chunk-pair granularity so the first matmuls only wait
                    # for the first 2 contraction chunks, not the full tile.
                    x8r = xt8.rearrange("(k p) t -> p k t", p=128)
                    xt8_sb = xtp.tile([128, NCH, TT], F8, tag="xts8",
                                      name=f"xts8_{j}")
                    for kk in range(0, NCH, 2):
                        nc.sync.dma_start(
                            out=xt8_sb[:, kk:kk + 2, :],
                            in_=x8r[:, kk:kk + 2, j * TT:(j + 1) * TT])
                    state["xt8"] = xt8_sb

                def do_load16():
                    xr = xt.rearrange("(k p) t -> p k t", p=128)
                    xt_sb = xtp.tile([128, NCH, TT], BF16, tag="xts",
                                     name=f"xts{j}")
                    for kk in range(0, NCH, 4):
                        nc.sync.dma_start(
                            out=xt_sb[:, kk:kk + 4, :],
                            in_=xr[:, kk:kk + 4, j * TT:(j + 1) * TT])
                    state["xt"] = xt_sb

                def do_qk(name, g):
                    # fp8e4 DoubleRow: two contraction chunks per matmul
                    pp = mixps.tile([128, TT], F32, tag="mix",
                                    name=f"pp_{name}{g}_{j}")
                    for kk in range(0, NCH, 2):
                        nc.tensor.matmul(
                            pp,
                            lhsT=w_sb[name][:, kk:kk + 2, g * 128:(g + 1) * 128],
                            rhs=state["xt8"][:, kk:kk + 2, :],
                            start=(kk == 0), stop=(kk == NCH - 2),
                            skip_group_check=True,
                            perf_mode=mybir.MatmulPerfMode.DoubleRow,
                        )
                    dst = QT if name == "q" else KT
                    nc.vector.tensor_scalar(
                        dst[:, g, j * TT:(j + 1) * TT], pp,
                        1.0 / WS, None, mybir.AluOpType.mult)

                def do_v(tb2):
                    vp = mixps.tile([128, TT], F32, tag="mix",
                                    name=f"vp{tb2}_{j}")
                    for half in range(2):
                        tl = tb2 * 2 + half
                        for kk in range(NCH):
                            nc.tensor.matmul(
                                vp[:, half * 256:(half + 1) * 256],
                                lhsT=state["xt"][:, kk, tl * 128:(tl + 1) * 128],
                                rhs=wv_sb[:, kk, :],
                                start=(kk == 0), stop=(kk == NCH - 1),
                                skip_group_check=True,
                            )
                    gtb = j * 4 + tb2 * 2
                    nc.vector.tensor_copy(
                        V[:, gtb:gtb + 2, :],
                        vp.rearrange("p (a b) -> p a b", a=2))

                ops = [lambda: do_qk("q", 0), lambda: do_qk("q", 1),
                       lambda: do_qk("k", 0), lambda: do_qk("k", 1),
                       lambda: do_v(0), lambda: do_v(1)]
                return do_load8, do_load16, ops

            def emit_attention(j, pending):
                n_sb = 4 * (j + 1)
                av_ps = {g: avps.tile([128, TT], F32, tag="avps",
                                      name=f"av{j}_{g}")
                         for g in range(NPAIR)}
                sm_ps = smps.tile([128, TT], F32, tag="smps", name=f"smps{j}")
                eg = {}
                LAG = 2

                def emit_scores(sb):
                    off = max(0, (sb - 4 * j) * SB)
                    for g in range(NPAIR):
                        S = spsum.tile([128, 2, TT], F32, tag="spsum",
                                       name=f"s{j}_{sb}_{g}")
                        for hh in range(2):
                            hp = slice(hh * 64, (hh + 1) * 64)
                            nc.tensor.matmul(
                                S[:, hh, off:],
                                lhsT=KT[hp, g, sb * SB:(sb + 1) * SB],
                                rhs=QT[hp, g, j * TT + off:(j + 1) * TT],
                                start=True, stop=True,
                            )
                        e = epool.tile([128, 2, TT], BF16, tag="e",
                                       name=f"e{j}_{sb}_{g}")
                        if sb != 0 and g != (sb & 1):
                            nc.vector.tensor_scalar(
                                e.bitcast(I16)[:, :, off:],
                                S[:, :, off:],
                                SCH_K, SCH_B,
                                mybir.AluOpType.mult, mybir.AluOpType.add)
                        else:
                            nc.scalar.activation(
                                out=e[:, :, off:], in_=S[:, :, off:],
                                func=mybir.ActivationFunctionType.Exp)
                        if sb >= 4 * j:  # diagonal block: causal triangle
                            nc.vector.tensor_mul(
                                e[:, :, off:off + SB],
                                e[:, :, off:off + SB], tri01)
                        eg[(sb, g)] = e

                def emit_av(sb):
                    off = max(0, (sb - 4 * j) * SB)
                    es = {g: eg.pop((sb, g)) for g in range(NPAIR)}
                    for g in range(NPAIR):
                        for hh in range(2):
                            h = 2 * g + hh
                            nc.tensor.matmul(
                                av_ps[g][hh * 64:(hh + 1) * 64, off:],
                                lhsT=V[:, sb, h * 64:(h + 1) * 64],
                                rhs=es[g][:, hh, off:],
                                start=(sb == 0), stop=(sb == n_sb - 1),
                                skip_group_check=True,
                            )
                    for g in range(NPAIR):
                        for hh in range(2):
                            h = 2 * g + hh
                            nc.tensor.matmul(
                                sm_ps[32 * h:32 * h + 1, off:],
                                lhsT=ones1,
                                rhs=es[g][:, hh, off:],
                                start=(sb == 0), stop=(sb == n_sb - 1),
                                skip_group_check=True,
                                tile_position=(0, 32 * h),
                            )

                n_periods = n_sb + LAG
                n_pend = len(pending)
                popped = 0
                # Drain all pending work BEFORE the tail periods: anything
                # queued between this tile's last scores and its final AV
                # matmuls delays the av stop -> delays the output drains ->
                # (via the ScalarE queue) stalls the next tile's exps and
                # lets the HAM clock-gate re-throttle the PE.
                ramp = max(n_sb - LAG - 1, 4)
                for sb in range(n_periods):
                    if sb < n_sb:
                        emit_scores(sb)
                    want = min(n_pend, (n_pend * (sb + 1)) // ramp)
                    # the first pending items are the next xt load and the
                    # PREVIOUS tile's output drains; force them out in the
                    # first two periods -- a not-yet-drained av/sums PSUM
                    # bank would stall this tile's first AV matmul at the
                    # head of the PE queue, blocking everything behind it.
                    if sb == 0:
                        want = max(want, min(n_pend, 4))
                    elif sb == 1:
                        want = max(want, min(n_pend, 5))
                    while popped < want:
                        pending[popped]()
                        popped += 1
                    if sb >= LAG:
                        emit_av(sb - LAG)
                assert popped == n_pend

                # Output drains, returned as closures and emitted early in
                # the NEXT tile's attention (after its first exp) so the
                # ScalarE copies don't sit ahead of that tile's first exps
                # in the queue. They run on ScalarE (idle vs DVE; closer to
                # PSUM).
                last = j == nj - 1

                def drain_y(g):
                    y_sb = ysbp.tile([128, TT], F32, tag="ysb",
                                     name=f"y{j}_{g}")
                    # final tile: split across engines to shorten the tail
                    if last and g == 1:
                        nc.vector.tensor_copy(y_sb, av_ps[g])
                    else:
                        nc.scalar.copy(y_sb, av_ps[g])
                    nc.sync.dma_start(
                        out=yt[g * 128:(g + 1) * 128, j * TT:(j + 1) * TT],
                        in_=y_sb)

                def drain_sm():
                    # sums live on strided partitions {0,32,64,96}; engine
                    # APs must start on 32-aligned partitions, so compact
                    # onto partition 0 along the free dim before the DMA.
                    sm_sb = smsbp.tile([1, HPC, TT], F32, tag="smsb",
                                       name=f"sm{j}")
                    for h in range(HPC):
                        if last and h % 2:
                            nc.vector.tensor_copy(
                                sm_sb[:, h, :], sm_ps[32 * h:32 * h + 1, :])
                        else:
                            nc.scalar.copy(
                                sm_sb[:, h, :], sm_ps[32 * h:32 * h + 1, :])
                    nc.sync.dma_start(out=sm[j:j + 1], in_=sm_sb)

                return [lambda: drain_y(0), lambda: drain_y(1), drain_sm]

            ld0_8, ld0_16, ops0 = proj_closures(0)
            ld0_8()
            nc.sync.dma_start(
                out=wq_sb, in_=wq.rearrange("(k p) d -> p k d", p=128))
            sm_ps_warm = smps.tile([128, TT], F32, tag="smps", name="smwarm")
            # PE warm-up: ~5us of dependency-free tiny matmuls during the
            # DMA fill so the HAM clock-gate is at 8/8 when the first
            # projection matmul issues (cold K=4/8 costs 2x for ~10us).
            # Writes scratch into the sums bank; attention later overwrites
            # it via start=True.
            for _ in range(240):
                nc.tensor.matmul(sm_ps_warm[0:1, 0:1], lhsT=ones1,
                                 rhs=ones1, start=True, stop=True,
                                 skip_group_check=True)
            # DMA ring order: fp8 xt (done), wq, wk, THEN the bf16 xt and
            # wv -- q/k proj (which gates the fill) only waits on the first
            # 2.5MB; the v path streams in behind it.
            nc.sync.dma_start(
                out=wk_sb, in_=wk.rearrange("(k p) d -> p k d", p=128))
            ops0[0]()
            ops0[1]()
            ld0_16()
            ops0[2]()
            ops0[3]()
            nc.sync.dma_start(
                out=wv_sb, in_=wv.rearrange("(k p) d -> p k d", p=128))
            ops0[4]()
            ops0[5]()
            drains = []
            for j in range(nj):
                if j + 1 < nj:
                    ld8n, ld16n, opsn = proj_closures(j + 1)
                    nxt = [ld8n, ld16n] + drains + opsn
                else:
                    nxt = list(drains)
                drains = emit_attention(j, nxt)
            for dr in drains:
                dr()

    nc.compile()
    return nc


_CACHE = {}


def _get_runner():
    if "run" in _CACHE:
        return _CACHE["run"]

    import jax
    from jax.experimental.shard_map import shard_map
    from jax.sharding import Mesh, PartitionSpec
    from concourse import bass2jax
    from concourse.bass2jax import _bass_exec_p, install_neuronx_cc_hook

    nc = build_nc()
    install_neuronx_cc_hook()

    partition_name = (nc.partition_id_tensor.name
                      if nc.partition_id_tensor else None)
    in_names, out_names, out_avals, zero_outs = [], [], [], []
    for alloc in nc.m.functions[0].allocations:
        if not isinstance(alloc, mybir.MemoryLocationSet):
            continue
        name = alloc.memorylocations[0].name
        if alloc.kind == "ExternalInput":
            if name != partition_name:
                in_names.append(name)
        elif alloc.kind == "ExternalOutput":
            out_names.append(name)
            shape = tuple(alloc.tensor_shape)
            dtype = mybir.dt.np(alloc.dtype)
            out_avals.append(jax.core.ShapedArray(shape, dtype))
            zero_outs.append(np.zeros(shape, dtype))
    n_params = len(in_names)
    n_outs = len(out_avals)
    all_names = in_names + out_names
    if partition_name is not None:
        all_names = all_names + [partition_name]
    donate = tuple(range(n_params, n_params + n_outs))

    def _body(*args):
        operands = list(args)
        if partition_name is not None:
            operands.append(bass2jax.partition_id_tensor())
        outs = _bass_exec_p.bind(
            *operands,
            out_avals=tuple(out_avals),
            in_names=tuple(all_names),
            out_names=tuple(out_names),
            lowering_input_output_aliases=(),
            sim_require_finite=True,
            sim_require_nnan=True,
            nc=nc,
        )
        return tuple(outs)

    devices = jax.devices()[:NCORES]
    mesh = Mesh(np.asarray(devices), ("core",))
    in_specs = (PartitionSpec("core"),) * (n_params + n_outs)
    out_specs = (PartitionSpec("core"),) * n_outs
    sharded = jax.jit(
        shard_map(_body, mesh=mesh, in_specs=in_specs, out_specs=out_specs,
                  check_rep=False),
        donate_argnums=donate, keep_unused=True,
    )

    runner = {
        "nc": nc,
        "all_names": all_names,
        "sharded": sharded,
        "in_names": in_names,
        "out_names": out_names,
        "out_avals": out_avals,
        "zero_outs": zero_outs,
    }
    _CACHE["run"] = runner
    return runner


def _shard_inputs(x, Wq, Wk, Wv):
    """Per-core input dicts. Host-side layout prep only."""
    bf = mybir.dt.np(BF16)
    f8 = mybir.dt.np(F8)
    scale = float(C) ** -0.5
    maps = []
    for c in range(NCORES):
        b, hg = divmod(c, 4)
        hs = list(range(HPC * hg, HPC * hg + HPC))
        xtb = np.ascontiguousarray(np.transpose(x[b]))  # [C, T]
        wq2 = np.ascontiguousarray(
            (np.concatenate([Wq[h] for h in hs], axis=1)
             * (scale * WS)).astype(f8))
        wk2 = np.ascontiguousarray(
            (np.concatenate([Wk[h] for h in hs], axis=1) * WS).astype(f8))
        wv2 = np.ascontiguousarray(
            np.concatenate([Wv[h] for h in hs], axis=1).astype(bf))
        maps.append({"xt": xtb.astype(bf), "xt8": xtb.astype(f8),
                     "wq": wq2, "wk": wk2, "wv": wv2})
    return maps


def run_sharded(in_maps):
    """Run the 8-core NEFF once; returns list of per-core output dicts."""
    r = _get_runner()
    concat_in = [
        np.concatenate([in_maps[c][name] for c in range(NCORES)], axis=0)
        for name in r["in_names"]
    ]
    concat_zeros = [
        np.zeros((NCORES * z.shape[0], *z.shape[1:]), z.dtype)
        for z in r["zero_outs"]
    ]
    out_arrs = r["sharded"](*concat_in, *concat_zeros)
    return [
        {
            name: np.asarray(out_arrs[i]).reshape(
                NCORES, *r["out_avals"][i].shape)[c]
            for i, name in enumerate(r["out_names"])
        }
        for c in range(NCORES)
    ]


def kernel(x, Wq, Wk, Wv):
    x = np.asarray(x, dtype=np.float32)
    Wq = np.asarray(Wq, dtype=np.float32)
    Wk = np.asarray(Wk, dtype=np.float32)
    Wv = np.asarray(Wv, dtype=np.float32)
    in_maps = _shard_inputs(x, Wq, Wk, Wv)
    results = run_sharded(in_maps)
    outs = []
    for b in range(B):
        parts = []
        for hg in range(4):
            r = results[b * 4 + hg]
            ytc = np.asarray(r["yt"], dtype=np.float32)   # [256, T]
            smc = np.asarray(r["sm"], dtype=np.float32)   # [nj, 4, TT]
            smc = smc.transpose(1, 0, 2).reshape(HPC, T)  # [4, T]
            yn = ytc.reshape(HPC, D, T) / smc[:, None, :]
            parts.append(yn.reshape(HPC * D, T).T)        # [T, 256]
        outs.append(np.concatenate(parts, axis=1))        # [T, 1024]
    return np.ascontiguousarray(np.stack(outs)).astype(np.float32)
